# revision 45
# baseline (speedup 1.0000x reference)
"""GTLayer (relational graph transformer layer) on 8 Trainium2 NeuronCores.

Strategy
--------
Nodes are partitioned across 8 cores in graph-aligned contiguous slices
(graphNorm stays core-local). Edges live with the core that owns dst.
Per core, dst nodes are processed in 256-node windows.

- graphNorm1: slice-local stats (one-pass sum/sumsq via one-hot matmuls),
  normalize, then AllGather hn -> global gather table (bf16).
- RelConv (Q|K|V fused, 384 cols): edges sorted by (window, src-half,
  relation), each (w,half,r) run padded to 128-slot chunks (>=1 pad slot per
  chunk, index 0, key -1). hn[src] rows fetched with the ext-isa dma_gather
  (int16 indices, signed, two base offsets cover the global table).
  Aggregation is one-hot matmuls in bf16 (PSUM accumulates f32).
- Attention: same chunk machinery per (window, half), all matmuls bf16.
- Epilogue per 128 rows: attn = wV/(z+eps), hO = attn@o_w+o_b, LN1,
  graphNorm2 (stats pass over SBUF-resident h1), FFN, LN2 -> output slice.

All per-core variation is in input data (indices/keys); the SPMD program is
identical across cores (chunk counts are max'ed over cores).
"""
import os
import sys
import types
import numpy as np
BF16 = np.float16

NCORES = 8
N_NODES = 100000
N_EDGES = 600000
D = 128
REL = 9
NG = 64
HEADS = 8
DH = 16
WIN = 256          # dst window (2 x 128 subwindows)
GMAX = 16          # max graphs per core
CALL_MAX = 8       # max chunks (of 128 slots) per dma_gather call


def _ensure_hooks():
    if "antenv.axon_hooks" not in sys.modules:
        hooks = types.ModuleType("antenv.axon_hooks")
        h = [None]
        hooks.set_axon_ntff_profile_hook = lambda v: h.__setitem__(0, v)
        hooks.get_axon_ntff_profile_hook = lambda: h[0]
        sys.modules["antenv.axon_hooks"] = hooks
        try:
            from trn_agent_boot.trn_boot import _ntff_profile_via_ctypes
            hooks.set_axon_ntff_profile_hook(
                _ntff_profile_via_ctypes("/opt/axon/libaxon_pjrt.so"))
        except Exception:
            pass


# ----------------------------------------------------------------------------
# Host preprocessing
# ----------------------------------------------------------------------------

def _pack_idx16(idx):
    """int16 index array -> [128, n/16] wrapped+replicated layout."""
    n = len(idx)
    assert n % 16 == 0
    blk = idx.reshape(n // 16, 16).T
    return np.tile(blk, (8, 1)).astype(np.int16)


def _layout_slots(order_edges, idx_vals, key_vals, n_chunks):
    """Place edges into n_chunks*128 slots, <=127 real per chunk, last slot of
    each chunk is a pad (idx 0, key -1). Returns (idx int32, key int16)."""
    tot = n_chunks * 128
    idx = np.zeros(tot, np.int32)
    key = np.full(tot, -1, np.int16)
    ne = len(order_edges)
    pos = 0
    ei = 0
    for c in range(n_chunks):
        take = min(127, ne - ei)
        if take > 0:
            sl = slice(c * 128, c * 128 + take)
            idx[sl] = idx_vals[order_edges[ei:ei + take]]
            key[sl] = key_vals[order_edges[ei:ei + take]]
            ei += take
    assert ei == ne, (ei, ne, n_chunks)
    return idx, key


def preprocess(inputs):
    h = np.asarray(inputs['h'], np.float32)
    src = np.asarray(inputs['src']).astype(np.int64)
    dst = np.asarray(inputs['dst']).astype(np.int64)
    et = np.asarray(inputs['etypes']).astype(np.int64)
    seg = np.asarray(inputs['seg']).astype(np.int64)

    # --- graph-aligned node partition ---
    gstart = np.searchsorted(seg, np.arange(NG + 1))  # graph g: [gstart[g], gstart[g+1])
    bounds = [0]
    for c in range(1, NCORES):
        target = c * N_NODES / NCORES
        g = int(np.argmin(np.abs(gstart - target)))
        bounds.append(int(gstart[g]))
    bounds.append(N_NODES)
    n0 = np.array(bounds[:-1]); n1 = np.array(bounds[1:])
    sizes = n1 - n0
    P_NODES = int(np.ceil(sizes.max() / WIN) * WIN)
    NW = P_NODES // WIN
    # lo/hi half-tables (each AllGathered separately so the lo collective can
    # overlap with compute of the hi half). int16 gather reach: rows/2 <= 32768.
    M_LO = (NW // 2) * WIN
    M_HI = P_NODES - M_LO
    ROWS_LO = NCORES * M_LO
    ROWS_HI = NCORES * M_HI
    assert ROWS_LO // 2 <= 32768 and ROWS_HI // 2 <= 32768

    owner = np.searchsorted(n1, np.arange(N_NODES), side='right')
    off_all = np.arange(N_NODES) - n0[owner]

    # --- per-core graph info ---
    g0 = np.searchsorted(gstart, n0, side='right') - 1  # first graph on core
    counts_g = np.diff(gstart).astype(np.float32)

    owner_s = owner[src]
    off_s = off_all[src]
    half = (off_s >= M_LO).astype(np.int64)
    ecore = owner[dst]
    dst_off = dst - n0[ecore]
    w_e = dst_off // WIN
    dl_e = (dst_off % WIN).astype(np.float32)
    idx_rel = np.where(half == 0,
                       owner_s * M_LO + off_s - ROWS_LO // 2,
                       owner_s * M_HI + (off_s - M_LO) - ROWS_HI // 2).astype(np.int32)

    # --- relconv structure: runs (w, half); slots sorted by key=et*256+dl so
    # chunks span relation boundaries (one matmul per (chunk, rel-present)) ---
    ckey = (et * 256 + dst_off % WIN).astype(np.int16)
    rkey = w_e * 2 + half
    rc_counts = np.zeros((NCORES, NW * 2), np.int64)
    for c in range(NCORES):
        m = ecore == c
        rc_counts[c] = np.bincount(rkey[m], minlength=NW * 2)
    rc_chunks = np.ceil(rc_counts / 127.0).max(0).astype(np.int64)

    # --- attention structure: segments (sw128, half) ---
    sw_e = dst_off // 128
    dl128 = (dst_off % 128).astype(np.int16)
    NSW = NW * 2
    akey = sw_e * 2 + half
    at_counts = np.zeros((NCORES, NSW * 2), np.int64)
    for c in range(NCORES):
        m = ecore == c
        at_counts[c] = np.bincount(akey[m], minlength=NSW * 2)
    at_chunks = np.maximum(np.ceil(at_counts / 127.0).max(0), 1).astype(np.int64)

    RC_CHUNKS = int(rc_chunks.sum())
    AT_CHUNKS = int(at_chunks.sum())

    # --- per-core data arrays ---
    # rc_sched[chunk] = tuple of relations present in that chunk on ANY core
    rc_rels = [set() for _ in range(RC_CHUNKS)]
    in_maps = []
    for c in range(NCORES):
        m = np.nonzero(ecore == c)[0]
        rk = rkey[m]
        order = np.argsort(rk * 4096 + ckey[m].astype(np.int64), kind='stable')
        edges_sorted = m[order]
        rk_sorted = rk[order]
        run_start = np.searchsorted(rk_sorted, np.arange(NW * 2))
        run_end = np.searchsorted(rk_sorted, np.arange(NW * 2) + 1)

        rc_idx = np.zeros(RC_CHUNKS * 128, np.int32)
        rc_key = np.full(RC_CHUNKS * 128, -1, np.int16)
        coff = 0
        for q in range(NW * 2):
            nch = int(rc_chunks[q])
            eidx = edges_sorted[run_start[q]:run_end[q]]
            ii, kk = _layout_slots(eidx, idx_rel, ckey, nch)
            rc_idx[coff * 128:(coff + nch) * 128] = ii
            rc_key[coff * 128:(coff + nch) * 128] = kk
            for k in range(nch):
                ee = eidx[k * 127:(k + 1) * 127]
                rc_rels[coff + k].update(int(x) for x in np.unique(et[ee]))
            coff += nch
        assert coff == RC_CHUNKS

        ak = akey[m]
        aorder = np.argsort(ak, kind='stable')
        aedges = m[aorder]
        ak_sorted = ak[aorder]
        astart = np.searchsorted(ak_sorted, np.arange(NSW * 2))
        aend = np.searchsorted(ak_sorted, np.arange(NSW * 2) + 1)
        at_idx = np.zeros(AT_CHUNKS * 128, np.int32)
        at_key = np.full(AT_CHUNKS * 128, -1, np.int16)
        coff = 0
        for q in range(NSW * 2):
            nch = int(at_chunks[q])
            eidx = aedges[astart[q]:aend[q]]
            ii, kk = _layout_slots(eidx, idx_rel, dl128, nch)
            at_idx[coff * 128:(coff + nch) * 128] = ii
            at_key[coff * 128:(coff + nch) * 128] = kk
            coff += nch
        assert coff == AT_CHUNKS

        hs = np.zeros((P_NODES, D), np.float32)
        hs[:sizes[c]] = h[n0[c]:n1[c]]
        segl = np.full(P_NODES, -1.0, np.float32)
        segl[:sizes[c]] = (seg[n0[c]:n1[c]] - g0[c]).astype(np.float32)
        ginc = np.zeros((GMAX, 1), np.float32)
        ng_c = int(seg[n1[c] - 1] - g0[c]) + 1
        assert ng_c <= GMAX
        ginc[:ng_c, 0] = 1.0 / counts_g[g0[c]:g0[c] + ng_c]

        im = {
            'h_slice': hs.astype(BF16),
            'seg_col': segl.reshape(NW * 2, 128).T.copy(),   # [128, NW*2]
            'inv_cnt': ginc,
            'rc_idx': _pack_idx16(rc_idx.astype(np.int16)),
            'rc_key': rc_key.reshape(RC_CHUNKS, 128).T.copy(),  # [128, RC_CHUNKS]
            'at_idx': _pack_idx16(at_idx.astype(np.int16)),
            'at_key': at_key.reshape(AT_CHUNKS, 128).T.copy(),
        }
        in_maps.append(im)

    # --- shared weights (same for all cores) ---
    def A(x):
        return np.ascontiguousarray(np.asarray(x, np.float32))
    Wrel = np.concatenate([
        np.einsum('rb,bio->rio', A(inputs[f'{nm}_coeff']), A(inputs[f'{nm}_basis']))
        for nm in ('q', 'k', 'v')], axis=2)            # [9, 128, 384]
    w_shared = {
        'w_rel': A(Wrel.reshape(REL * D, 3 * D)).astype(BF16),
        'w_loop': np.concatenate([A(inputs[f'{nm}_loop']) for nm in ('q', 'k', 'v')], 1).astype(BF16),
        'b_qkv': np.tile(np.concatenate([A(inputs[f'{nm}_bias']) for nm in ('q', 'k', 'v')])[None, :], (128, 1)),
        'o_w': A(inputs['o_w']).astype(BF16), 'o_b': np.tile(A(inputs['o_b'])[None, :], (128, 1)),
        'ffn1': A(inputs['ffn1_w']).astype(BF16), 'ffn1_b': np.tile(A(inputs['ffn1_b'])[None, :], (128, 1)),
        'ffn2': A(inputs['ffn2_w']).astype(BF16), 'ffn2_b': np.tile(A(inputs['ffn2_b'])[None, :], (128, 1)),
        'ln1_g': np.tile(A(inputs['ln1_g'])[None, :], (128, 1)),
        'ln1_b': np.tile(A(inputs['ln1_b'])[None, :], (128, 1)),
        'ln2_g': np.tile(A(inputs['ln2_g'])[None, :], (128, 1)),
        'ln2_b': np.tile(A(inputs['ln2_b'])[None, :], (128, 1)),
    }
    for nm in ('gn1', 'gn2'):
        w = A(inputs[f'{nm}_w']); b = A(inputs[f'{nm}_b']); ms = A(inputs[f'{nm}_ms'])
        w_shared[f'{nm}_w16'] = np.tile(w[None, :], (GMAX, 1))
        w_shared[f'{nm}_b16'] = np.tile(b[None, :], (GMAX, 1))
        w_shared[f'{nm}_ms16'] = np.tile(ms[None, :], (GMAX, 1))
        w_shared[f'{nm}_msfac16'] = np.tile((ms * (2 - ms))[None, :], (GMAX, 1))
    for im in in_maps:
        im.update(w_shared)

    static = dict(P_NODES=P_NODES, NW=NW, M_LO=M_LO, M_HI=M_HI,
                  rc_chunks=tuple(int(x) for x in rc_chunks),
                  rc_sched=tuple(tuple(sorted(s)) for s in rc_rels),
                  at_chunks=tuple(int(x) for x in at_chunks),
                  RC_CHUNKS=RC_CHUNKS, AT_CHUNKS=AT_CHUNKS)
    meta = dict(n0=n0, n1=n1, sizes=sizes)
    return static, in_maps, meta


# ----------------------------------------------------------------------------
# Bass program
# ----------------------------------------------------------------------------

_PROGRAM_CACHE = {}


def build_program(st):
    import concourse.bass as bass
    import concourse.bacc as bacc
    import concourse.mybir as mybir
    import concourse.tile as tile
    from concourse.tile import TileContext
    from concourse.masks import make_identity
    from bass_rust import add_dep_helper

    P_NODES = st['P_NODES']; NW = st['NW']
    M_LO = st['M_LO']; M_HI = st['M_HI']
    ROWS_LO = NCORES * M_LO; ROWS_HI = NCORES * M_HI
    rc_chunks = st['rc_chunks']; at_chunks = st['at_chunks']
    rc_sched = st['rc_sched']
    RC_CHUNKS = st['RC_CHUNKS']; AT_CHUNKS = st['AT_CHUNKS']
    f32 = mybir.dt.float32
    bf16 = mybir.dt.float16
    i16 = mybir.dt.int16
    AO = mybir.AluOpType
    AF = mybir.ActivationFunctionType

    nc = bacc.Bacc()

    # --- I/O ---
    h_slice = nc.declare_dram_parameter('h_slice', [P_NODES, D], bf16, isOutput=False)
    seg_col = nc.declare_dram_parameter('seg_col', [128, NW * 2], f32, isOutput=False)
    inv_cnt = nc.declare_dram_parameter('inv_cnt', [GMAX, 1], f32, isOutput=False)
    rc_idx = nc.declare_dram_parameter('rc_idx', [128, RC_CHUNKS * 8], i16, isOutput=False)
    rc_keyd = nc.declare_dram_parameter('rc_key', [128, RC_CHUNKS], i16, isOutput=False)
    at_idx = nc.declare_dram_parameter('at_idx', [128, AT_CHUNKS * 8], i16, isOutput=False)
    at_keyd = nc.declare_dram_parameter('at_key', [128, AT_CHUNKS], i16, isOutput=False)
    w_rel = nc.declare_dram_parameter('w_rel', [REL * D, 3 * D], bf16, isOutput=False)
    w_loop = nc.declare_dram_parameter('w_loop', [D, 3 * D], bf16, isOutput=False)
    b_qkv = nc.declare_dram_parameter('b_qkv', [128, 3 * D], f32, isOutput=False)
    o_w = nc.declare_dram_parameter('o_w', [D, D], bf16, isOutput=False)
    o_b = nc.declare_dram_parameter('o_b', [128, D], f32, isOutput=False)
    ffn1 = nc.declare_dram_parameter('ffn1', [D, 2 * D], bf16, isOutput=False)
    ffn1_b = nc.declare_dram_parameter('ffn1_b', [128, 2 * D], f32, isOutput=False)
    ffn2 = nc.declare_dram_parameter('ffn2', [2 * D, D], bf16, isOutput=False)
    ffn2_b = nc.declare_dram_parameter('ffn2_b', [128, D], f32, isOutput=False)
    cdecl = {}
    for nm in ('ln1_g', 'ln1_b', 'ln2_g', 'ln2_b'):
        cdecl[nm] = nc.declare_dram_parameter(nm, [128, D], f32, isOutput=False)
    for nm in ('gn1', 'gn2'):
        for sfx in ('w16', 'b16', 'ms16', 'msfac16'):
            cdecl[f'{nm}_{sfx}'] = nc.declare_dram_parameter(
                f'{nm}_{sfx}', [GMAX, D], f32, isOutput=False)
    out_sl = nc.declare_dram_parameter('out_slice', [P_NODES, D], f32, isOutput=True)

    # --- internal DRAM ---
    hn_local = nc.dram_tensor('hn_local', [P_NODES, D], bf16)
    q_local = nc.dram_tensor('q_local', [P_NODES, D], bf16)
    kv_local = nc.dram_tensor('kv_local', [P_NODES, 2 * D], bf16)
    debug = os.environ.get('KERNEL_DEBUG') == '1'
    if debug:
        hn_dbg = nc.declare_dram_parameter('hn_dbg', [P_NODES, D], bf16, isOutput=True)
        q_dbg = nc.declare_dram_parameter('q_dbg', [P_NODES, D], bf16, isOutput=True)
        kv_dbg = nc.declare_dram_parameter('kv_dbg', [P_NODES, 2 * D], bf16, isOutput=True)
    hn_lo = nc.dram_tensor('hn_lo', [NCORES, M_LO, D], bf16, addr_space='Shared')
    hn_hi = nc.dram_tensor('hn_hi', [NCORES, M_HI, D], bf16, addr_space='Shared')
    kv_lo = nc.dram_tensor('kv_lo', [NCORES, M_LO, 2 * D], bf16, addr_space='Shared')
    kv_hi = nc.dram_tensor('kv_hi', [NCORES, M_HI, 2 * D], bf16, addr_space='Shared')
    # gather base views: idx 0 points at the middle row of each half-table
    hn_tab = [hn_lo[:].rearrange('c p d -> (c p) d')[ROWS_LO // 2:, :],
              hn_hi[:].rearrange('c p d -> (c p) d')[ROWS_HI // 2:, :]]
    kv_tab = [kv_lo[:].rearrange('c p d -> (c p) d')[ROWS_LO // 2:, :],
              kv_hi[:].rearrange('c p d -> (c p) d')[ROWS_HI // 2:, :]]

    with TileContext(nc) as tc:
        with tc.tile_pool(name='const', bufs=1) as cpool:
            # constants
            iota2304 = cpool.tile([128, REL * WIN], i16)
            nc.gpsimd.iota(iota2304[:], pattern=[[1, REL * WIN]], base=0,
                           channel_multiplier=0)
            iotaG = cpool.tile([128, GMAX], f32)
            nc.gpsimd.iota(iotaG[:], pattern=[[1, GMAX]], base=0,
                           channel_multiplier=0, allow_small_or_imprecise_dtypes=True)
            iotaGG = cpool.tile([128, 2 * GMAX], f32)
            nc.gpsimd.iota(iotaGG[:].rearrange('p (c f) -> p c f', f=GMAX),
                           pattern=[[0, 2], [1, GMAX]], base=0,
                           channel_multiplier=0, allow_small_or_imprecise_dtypes=True)
            ident = cpool.tile([128, 128], bf16)
            make_identity(nc, ident[:])
            ones1 = cpool.tile([1, 128], f32)
            nc.gpsimd.memset(ones1[:], 1.0)
            neg5 = cpool.tile([128, 1], f32)
            nc.gpsimd.memset(neg5[:], -5.0)
            epsz = cpool.tile([128, 1], f32)
            nc.gpsimd.memset(epsz[:], 6.7379470e-09)   # 1e-6 * exp(-5)
            zeros2304 = cpool.tile([128, REL * WIN], f32)
            nc.gpsimd.memset(zeros2304[:], 0.0)
            eps5 = cpool.tile([128, 1], f32)
            nc.gpsimd.memset(eps5[:], 1e-5)
            invd = cpool.tile([128, 1], f32)
            nc.gpsimd.memset(invd[:], 1.0 / D)
            ones_d = cpool.tile([128, D], f32)
            nc.gpsimd.memset(ones_d[:], 1.0)
            iota256 = cpool.tile([128, 256], i16)       # j % 128 pattern
            nc.gpsimd.iota(iota256[:].rearrange('p (c f) -> p c f', f=128),
                           pattern=[[0, 2], [1, 128]], base=0,
                           channel_multiplier=0)
            c40h = cpool.tile([128, 1], bf16)
            nc.gpsimd.memset(c40h[:], 40.0)

            segs = cpool.tile([128, NW * 2], f32)
            nc.sync.dma_start(out=segs[:], in_=seg_col[:])
            rck = cpool.tile([128, RC_CHUNKS], i16)
            nc.sync.dma_start(out=rck[:], in_=rc_keyd[:])
            atk = cpool.tile([128, AT_CHUNKS], i16)
            nc.sync.dma_start(out=atk[:], in_=at_keyd[:])
            rci = cpool.tile([128, RC_CHUNKS * 8], i16)
            nc.sync.dma_start(out=rci[:], in_=rc_idx[:])
            ati = cpool.tile([128, AT_CHUNKS * 8], i16)
            nc.sync.dma_start(out=ati[:], in_=at_idx[:])

            wrel_sb = cpool.tile([128, REL * 3 * D], bf16)  # r-th block at [:, r*384:(r+1)*384]
            for r in range(REL):
                nc.sync.dma_start(out=wrel_sb[:, r * 3 * D:(r + 1) * 3 * D],
                                  in_=w_rel[r * D:(r + 1) * D, :])
            wloop_sb = cpool.tile([128, 3 * D], bf16)
            nc.sync.dma_start(out=wloop_sb[:], in_=w_loop[:])
            bqkv_sb = cpool.tile([128, 3 * D], f32)
            nc.sync.dma_start(out=bqkv_sb[:], in_=b_qkv[:])
            ow_sb = cpool.tile([D, D], bf16)
            nc.sync.dma_start(out=ow_sb[:], in_=o_w[:])
            ob_sb = cpool.tile([128, D], f32)
            nc.sync.dma_start(out=ob_sb[:], in_=o_b[:])
            ffn1_sb = cpool.tile([D, 2 * D], bf16)
            nc.sync.dma_start(out=ffn1_sb[:], in_=ffn1[:])
            ffn1b_sb = cpool.tile([128, 2 * D], f32)
            nc.sync.dma_start(out=ffn1b_sb[:], in_=ffn1_b[:])
            ffn2_sb = cpool.tile([128, 2 * D], bf16)  # two K-chunks side by side
            nc.sync.dma_start(out=ffn2_sb[:, :D], in_=ffn2[:D, :])
            nc.sync.dma_start(out=ffn2_sb[:, D:], in_=ffn2[D:, :])
            ffn2b_sb = cpool.tile([128, D], f32)
            nc.sync.dma_start(out=ffn2b_sb[:], in_=ffn2_b[:])
            csb = {}
            for nm, dd in cdecl.items():
                t = cpool.tile(list(dd.shape), f32, tag=f'c_{nm}')
                nc.sync.dma_start(out=t[:], in_=dd[:])
                csb[nm] = t
            invc_sb = cpool.tile([GMAX, 1], f32)
            nc.sync.dma_start(out=invc_sb[:], in_=inv_cnt[:])

            NSW = NW * 2  # number of 128-row subwindows

            # =========== phase 1: graphNorm1 ===========
            sc1 = nc.enter_named_scope('ph1_gn1', False)
            with (
                tc.tile_pool(name='p1keep', bufs=1) as keep1,
                tc.tile_pool(name='p1sb', bufs=3) as sb1,
                tc.tile_pool(name='p1ps', bufs=1, space='PSUM') as ps1,
                tc.tile_pool(name='p1ps2', bufs=1, space='PSUM') as ps1b,
            ):
                sum_ps = ps1.tile([GMAX, D], f32, tag='sums')
                sq_ps = ps1.tile([GMAX, D], f32, tag='sqs')
                hwins = []
                for sp in range(NSW // 2):
                    hw = keep1.tile([128, 2 * D], bf16, tag=f'h_{sp}')
                    nc.sync.dma_start(out=hw[:, :D],
                                      in_=h_slice[2 * sp * 128:(2 * sp + 1) * 128, :])
                    nc.sync.dma_start(out=hw[:, D:],
                                      in_=h_slice[(2 * sp + 1) * 128:(2 * sp + 2) * 128, :])
                    B2 = sb1.tile([128, 2 * GMAX], bf16, tag='B1')
                    nc.vector.tensor_tensor(
                        out=B2[:].rearrange('p (c g) -> p c g', g=GMAX),
                        in0=segs[:, 2 * sp:2 * sp + 2].rearrange('p (c o) -> p c o', o=1)
                            .to_broadcast([128, 2, GMAX]),
                        in1=iotaGG[:].rearrange('p (c g) -> p c g', g=GMAX),
                        op=AO.is_equal)
                    hsq = sb1.tile([128, 2 * D], bf16, tag='hsq')
                    nc.gpsimd.tensor_tensor(out=hsq[:], in0=hw[:], in1=hw[:], op=AO.mult)
                    for j in range(2):
                        s = 2 * sp + j
                        nc.tensor.matmul(out=sum_ps[:], lhsT=B2[:, j * GMAX:(j + 1) * GMAX],
                                         rhs=hw[:, j * D:(j + 1) * D],
                                         start=(s == 0), stop=(s == NSW - 1))
                        nc.tensor.matmul(out=sq_ps[:], lhsT=B2[:, j * GMAX:(j + 1) * GMAX],
                                         rhs=hsq[:, j * D:(j + 1) * D],
                                         start=(s == 0), stop=(s == NSW - 1))
                    hwins.append(hw)
                # finalize: alpha/beta [GMAX, D]
                mean = keep1.tile([GMAX, D], f32)
                nc.vector.tensor_tensor(out=mean[:], in0=sum_ps[:],
                                        in1=invc_sb[:].to_broadcast([GMAX, D]), op=AO.mult)
                ex2 = keep1.tile([GMAX, D], f32)
                nc.vector.tensor_tensor(out=ex2[:], in0=sq_ps[:],
                                        in1=invc_sb[:].to_broadcast([GMAX, D]), op=AO.mult)
                msq = keep1.tile([GMAX, D], f32)
                nc.vector.tensor_tensor(out=msq[:], in0=mean[:], in1=mean[:], op=AO.mult)
                nc.vector.tensor_tensor(out=msq[:], in0=msq[:], in1=csb['gn1_msfac16'][:], op=AO.mult)
                var = keep1.tile([GMAX, D], f32)
                nc.vector.tensor_tensor(out=var[:], in0=ex2[:], in1=msq[:], op=AO.subtract)
                nc.vector.tensor_scalar_add(out=var[:], in0=var[:], scalar1=1e-6)
                std = keep1.tile([GMAX, D], f32)
                nc.scalar.activation(out=std[:], in_=var[:], func=AF.Sqrt)
                rstd = keep1.tile([GMAX, D], f32)
                nc.vector.reciprocal(out=rstd[:], in_=std[:])
                alpha1 = keep1.tile([GMAX, D], f32)
                nc.vector.tensor_tensor(out=alpha1[:], in0=rstd[:], in1=csb['gn1_w16'][:], op=AO.mult)
                beta1 = keep1.tile([GMAX, D], f32)
                nc.vector.tensor_tensor(out=beta1[:], in0=mean[:], in1=csb['gn1_ms16'][:], op=AO.mult)
                nc.vector.tensor_tensor(out=beta1[:], in0=beta1[:], in1=alpha1[:], op=AO.mult)
                nc.vector.tensor_tensor(out=beta1[:], in0=csb['gn1_b16'][:], in1=beta1[:], op=AO.subtract)
                alpha1h = keep1.tile([GMAX, D], bf16)
                nc.vector.tensor_copy(out=alpha1h[:], in_=alpha1[:])
                beta1h = keep1.tile([GMAX, D], bf16)
                nc.vector.tensor_copy(out=beta1h[:], in_=beta1[:])
                # apply
                hn_stores = []
                SW_LO = M_LO // 128
                cc1 = [None, None]
                for sp in range(NSW // 2):
                    B2 = sb1.tile([128, 2 * GMAX], bf16, tag='B1b')
                    nc.vector.tensor_tensor(
                        out=B2[:].rearrange('p (c g) -> p c g', g=GMAX),
                        in0=segs[:, 2 * sp:2 * sp + 2].rearrange('p (c o) -> p c o', o=1)
                            .to_broadcast([128, 2, GMAX]),
                        in1=iotaGG[:].rearrange('p (c g) -> p c g', g=GMAX),
                        op=AO.is_equal)
                    bt_ps = ps1b.tile([GMAX, 256], bf16, tag='bt')
                    for j in range(2):
                        nc.tensor.transpose(out=bt_ps[:, j * 128:(j + 1) * 128],
                                            in_=B2[:, j * GMAX:(j + 1) * GMAX],
                                            identity=ident[:])
                    bt = sb1.tile([GMAX, 256], bf16, tag='btsb')
                    nc.scalar.activation(out=bt[:], in_=bt_ps[:], func=AF.Copy)
                    ab_ps = ps1b.tile([128, 4 * D], f32, tag='ab')
                    for j in range(2):
                        nc.tensor.matmul(out=ab_ps[:, j * 2 * D:j * 2 * D + D],
                                         lhsT=bt[:, j * 128:(j + 1) * 128],
                                         rhs=alpha1h[:], start=True, stop=False)
                        nc.tensor.matmul(out=ab_ps[:, j * 2 * D + D:(j + 1) * 2 * D],
                                         lhsT=bt[:, j * 128:(j + 1) * 128],
                                         rhs=beta1h[:], start=True, stop=True)
                    ab_v = ab_ps[:].rearrange('p (c x d) -> p c x d', x=2, d=D)
                    hnw = sb1.tile([128, 2 * D], bf16, tag='hnw')
                    hnv = hnw[:].rearrange('p (c d) -> p c d', d=D)
                    nc.vector.tensor_tensor(out=hnv,
                                            in0=hwins[sp][:].rearrange('p (c d) -> p c d', d=D),
                                            in1=ab_v[:, :, 0, :], op=AO.mult)
                    nc.vector.tensor_tensor(out=hnv, in0=hnv, in1=ab_v[:, :, 1, :], op=AO.add)
                    si0 = nc.sync.dma_start(out=hn_local[2 * sp * 128:(2 * sp + 1) * 128, :],
                                            in_=hnw[:, :D])
                    si1 = nc.sync.dma_start(out=hn_local[(2 * sp + 1) * 128:(2 * sp + 2) * 128, :],
                                            in_=hnw[:, D:])
                    hn_stores.extend([si0, si1])
                    if 2 * sp + 2 == SW_LO:
                        cc1[0] = nc.gpsimd.collective_compute(
                            'AllGather', AO.bypass, replica_groups=[list(range(NCORES))],
                            ins=[hn_local[0:M_LO, :]], outs=[hn_lo[:]])
                        for st_i in hn_stores:
                            add_dep_helper(cc1[0].ins, st_i.ins, True, 'cc1a after lo stores')
                cc1[1] = nc.gpsimd.collective_compute(
                    'AllGather', AO.bypass, replica_groups=[list(range(NCORES))],
                    ins=[hn_local[M_LO:, :]], outs=[hn_hi[:]])
                for st_i in hn_stores[SW_LO:]:
                    add_dep_helper(cc1[1].ins, st_i.ins, True, 'cc1b after hi stores')

            nc.leave_named_scope('ph1_gn1', sc1[0], False)

            # =========== phase 2: relconv (fused QKV) ===========
            sc2 = nc.enter_named_scope('ph2_relconv', False)
            with (
                tc.tile_pool(name='p2g', bufs=3) as gp2,
                tc.tile_pool(name='p2sb', bufs=3) as sb2,
                tc.tile_pool(name='p2S', bufs=1, space='PSUM') as psS,
                tc.tile_pool(name='p2qkv', bufs=1, space='PSUM') as psQ,
                tc.tile_pool(name='p2tr', bufs=1, space='PSUM') as psT,
            ):
                rc_off = 0   # chunk offset
                kv_stores = []
                cc2 = [None, None]
                W_LO = M_LO // WIN
                for w in range(NW):
                    qkv_ps = [psQ.tile([128, 3 * D], f32, tag=f'qkv{i}', name=f'qkv{i}') for i in range(2)]
                    S = psS.tile([128, REL * WIN], f32, tag='S', name='S')
                    # zero the whole S tile: regions are half-bank sized, so
                    # per-region matmul start=True cannot be used safely
                    nc.scalar.activation(out=S[:], in_=zeros2304[:], func=AF.Copy)
                    # (chunk, rel) touch schedule for this window: start/stop
                    # flags per relation region of S
                    touches = []
                    for hh in range(2):
                        nch = rc_chunks[w * 2 + hh]
                        for k in range(nch):
                            for r in rc_sched[rc_off + k]:
                                touches.append((hh, k, r))
                        rc_off += nch
                    rc_off -= rc_chunks[w * 2] + rc_chunks[w * 2 + 1]
                    first_touch = {}
                    last_touch = {}
                    for t in touches:
                        r = t[2]
                        if r not in first_touch:
                            first_touch[r] = t
                        last_touch[r] = t
                    rels_present = sorted(first_touch)
                    for hh in range(2):
                        nch = rc_chunks[w * 2 + hh]
                        co = rc_off
                        gtiles = {}
                        done = 0
                        while done < nch:
                            take = min(nch - done, CALL_MAX)
                            gt = gp2.tile([128, CALL_MAX * D], bf16, tag='g')
                            gi = nc.gpsimd.dma_gather(
                                out_ap=gt[:, :take * D].rearrange('p (c e) -> p c e', e=D),
                                in_ap=hn_tab[hh],
                                idxs_ap=rci[:, (co + done) * 8:(co + done + take) * 8],
                                num_idxs=take * 128, num_idxs_reg=take * 128,
                                elem_size=D)
                            add_dep_helper(gi.ins, cc1[hh].ins, True,
                                           'gather reads allgathered hn')
                            for j in range(take):
                                gtiles[done + j] = (gt, j)
                            done += take
                        for k in range(nch):
                            gt, j = gtiles[k]
                            rels = rc_sched[rc_off + k]
                            if not rels:
                                continue
                            rmin, rmax = rels[0], rels[-1]
                            span = rmax - rmin + 1
                            # one is_eq covers the chunk's whole relation span
                            A = sb2.tile([128, REL * WIN], bf16, tag='A')
                            nc.vector.tensor_tensor(
                                out=A[:, :span * WIN],
                                in0=rck[:, rc_off + k:rc_off + k + 1].to_broadcast([128, span * WIN]),
                                in1=iota2304[:, rmin * WIN:(rmax + 1) * WIN], op=AO.is_equal)
                            for r in rels:
                                nc.tensor.matmul(
                                    out=S[:, r * WIN:(r + 1) * WIN],
                                    lhsT=gt[:, j * D:(j + 1) * D],
                                    rhs=A[:, (r - rmin) * WIN:(r - rmin + 1) * WIN],
                                    start=False,
                                    stop=(last_touch[r] == (hh, k, r)),
                                    skip_group_check=True)
                        rc_off += nch
                    # transforms: batched PSUM->SBUF casts, then per-rel matmuls
                    st_all = sb2.tile([128, REL * WIN], bf16, tag='St')
                    for piece in range(3):
                        nc.vector.tensor_copy(
                            out=st_all[:, piece * 3 * WIN:(piece + 1) * 3 * WIN],
                            in_=S[:, piece * 3 * WIN:(piece + 1) * 3 * WIN])
                    for sub in range(2):
                        for ri, r in enumerate(rels_present):
                            nc.tensor.matmul(out=qkv_ps[sub][:],
                                             lhsT=st_all[:, r * WIN + sub * 128:r * WIN + sub * 128 + 128],
                                             rhs=wrel_sb[:, r * 3 * D:(r + 1) * 3 * D],
                                             start=(ri == 0), stop=False)
                    # self-loop + bias + relu + store
                    for sub in range(2):
                        row0 = w * WIN + sub * 128
                        hnw = sb2.tile([128, D], bf16, tag='hnl')
                        nc.sync.dma_start(out=hnw[:], in_=hn_local[row0:row0 + 128, :])
                        ht_ps = psT.tile([128, 128], bf16, tag='ht')
                        nc.tensor.transpose(out=ht_ps[:], in_=hnw[:], identity=ident[:])
                        ht = sb2.tile([128, 128], bf16, tag='htsb')
                        nc.vector.tensor_copy(out=ht[:], in_=ht_ps[:])
                        nc.tensor.matmul(out=qkv_ps[sub][:], lhsT=ht[:], rhs=wloop_sb[:],
                                         start=(len(rels_present) == 0), stop=True)
                        qkv_sb = sb2.tile([128, 3 * D], bf16, tag='qkvsb')
                        nc.vector.tensor_tensor(out=qkv_sb[:], in0=qkv_ps[sub][:],
                                                in1=bqkv_sb[:], op=AO.add)
                        nc.scalar.activation(out=qkv_sb[:], in_=qkv_sb[:], func=AF.Relu)
                        nc.sync.dma_start(out=q_local[row0:row0 + 128, :], in_=qkv_sb[:, :D])
                        si = nc.sync.dma_start(out=kv_local[row0:row0 + 128, :], in_=qkv_sb[:, D:])
                        kv_stores.append(si)
                    if w == W_LO - 1:
                        cc2[0] = nc.gpsimd.collective_compute(
                            'AllGather', AO.bypass, replica_groups=[list(range(NCORES))],
                            ins=[kv_local[0:M_LO, :]], outs=[kv_lo[:]])
                        for st_i in kv_stores:
                            add_dep_helper(cc2[0].ins, st_i.ins, True, 'cc2a after lo kv stores')
                cc2[1] = nc.gpsimd.collective_compute(
                    'AllGather', AO.bypass, replica_groups=[list(range(NCORES))],
                    ins=[kv_local[M_LO:, :]], outs=[kv_hi[:]])
                for st_i in kv_stores[2 * W_LO:]:
                    add_dep_helper(cc2[1].ins, st_i.ins, True, 'cc2b after hi kv stores')

            nc.leave_named_scope('ph2_relconv', sc2[0], False)

            # =========== phase 3: attention + epilogue ===========
            sc3 = nc.enter_named_scope('ph3_attn', False)
            NCH_MAX = max(at_chunks[2 * s] + at_chunks[2 * s + 1] for s in range(NSW))
            h1_cm = tc.tile_pool(name='h1', bufs=1)
            h1_pool = h1_cm.__enter__()
            h1t = []
            with (
                tc.tile_pool(name='p3g', bufs=3) as gp3,
                tc.tile_pool(name='p3sb', bufs=3) as sb3,
                tc.tile_pool(name='p3at', bufs=2, space='PSUM') as psA,
                tc.tile_pool(name='p3wv', bufs=2, space='PSUM') as psW,
                tc.tile_pool(name='p3ep', bufs=1, space='PSUM') as psE,
            ):
                at_off = 0
                for sw in range(NSW):
                    qwin = sb3.tile([128, D], bf16, tag='qwin')
                    nc.sync.dma_start(out=qwin[:], in_=q_local[sw * 128:(sw + 1) * 128, :])
                    wvz = psW.tile([128, D + HEADS], f32, tag='wvz')
                    nl = at_chunks[sw * 2]
                    nh = at_chunks[sw * 2 + 1]
                    nch = nl + nh
                    # one contiguous gather region per subwindow: lo chunks
                    # [0, nl) then hi chunks [nl, nch)
                    gt = gp3.tile([128, NCH_MAX * 2 * D], bf16, tag='ag')
                    for hh, cnt, coff in ((0, nl, 0), (1, nh, nl)):
                        done = 0
                        while done < cnt:
                            take = min(cnt - done, CALL_MAX)
                            gi = nc.gpsimd.dma_gather(
                                out_ap=gt[:, (coff + done) * 2 * D:(coff + done + take) * 2 * D]
                                    .rearrange('p (c e) -> p c e', e=2 * D),
                                in_ap=kv_tab[hh],
                                idxs_ap=ati[:, (at_off + coff + done) * 8:(at_off + coff + done + take) * 8],
                                num_idxs=take * 128, num_idxs_reg=take * 128,
                                elem_size=2 * D)
                            add_dep_helper(gi.ins, cc2[hh].ins, True,
                                           'gather reads allgathered kv')
                            done += take
                    # process chunks in pairs (last one may be a singleton)
                    p = 0
                    first = True
                    while p < nch:
                        cn = 2 if p + 1 < nch else 1
                        ck = at_off + p
                        A2 = sb3.tile([128, 256], bf16, tag='aA')
                        nc.vector.tensor_tensor(
                            out=A2[:, :cn * 128].rearrange('p (c f) -> p c f', f=128),
                            in0=atk[:, ck:ck + cn].rearrange('p (c o) -> p c o', o=1)
                                .to_broadcast([128, cn, 128]),
                            in1=iota256[:, :cn * 128].rearrange('p (c f) -> p c f', f=128),
                            op=AO.is_equal)
                        at_ps = psA.tile([128, 256], bf16, tag='atp')
                        for j in range(cn):
                            nc.tensor.transpose(out=at_ps[:, j * 128:(j + 1) * 128],
                                                in_=A2[:, j * 128:(j + 1) * 128],
                                                identity=ident[:])
                        att2 = sb3.tile([128, 256], bf16, tag='att')
                        nc.vector.tensor_copy(out=att2[:, :cn * 128], in_=at_ps[:, :cn * 128])
                        qd_ps = psA.tile([128, 256], f32, tag='qd')
                        for j in range(cn):
                            nc.tensor.matmul(out=qd_ps[:, j * D:(j + 1) * D],
                                             lhsT=att2[:, j * 128:(j + 1) * 128],
                                             rhs=qwin[:], start=True, stop=True)
                        kv2 = gt[:, p * 2 * D:(p + cn) * 2 * D].rearrange(
                            'p (c z) -> p c z', z=2 * D)
                        kq2 = sb3.tile([128, 256], f32, tag='kq')
                        nc.vector.tensor_tensor(
                            out=kq2[:, :cn * D].rearrange('p (c d) -> p c d', d=D),
                            in0=kv2[:, :, :D],
                            in1=qd_ps[:, :cn * D].rearrange('p (c d) -> p c d', d=D),
                            op=AO.mult)
                        sc2 = sb3.tile([128, 2 * HEADS], bf16, tag='sc')
                        with nc.allow_low_precision(reason='16-elem head dot, fp16 ok'):
                            nc.vector.reduce_sum(
                                out=sc2[:, :cn * HEADS].rearrange('p (g o) -> p g o', o=1),
                                in_=kq2[:, :cn * D].rearrange('p (g e) -> p g e', e=DH),
                                axis=mybir.AxisListType.X)
                        # scores are >= 0 (dot of relu vectors): only the upper
                        # clip can bind: min(s,40)/4 == clip(s/4, 10). The /4
                        # folds into the exp scale; exp is shifted by -5 so
                        # V*exp fits fp16 (wV and z scale together).
                        nc.vector.tensor_tensor(out=sc2[:, :cn * HEADS],
                                                in0=sc2[:, :cn * HEADS],
                                                in1=c40h[:].to_broadcast([128, cn * HEADS]),
                                                op=AO.min)
                        vse2 = sb3.tile([128, 2 * (D + HEADS)], bf16, tag='vse')
                        vv = vse2[:].rearrange('p (c x) -> p c x', x=D + HEADS)
                        nc.scalar.activation(
                            out=vv[:, :cn, D:], in_=sc2[:, :cn * HEADS]
                                .rearrange('p (c h) -> p c h', h=HEADS),
                            func=AF.Exp, bias=neg5[:], scale=0.25)
                        nc.vector.tensor_tensor(
                            out=vv[:, :cn, :D].rearrange('p c (h e) -> p c h e', e=DH),
                            in0=kv2[:, :, D:].rearrange('p c (h e) -> p c h e', e=DH),
                            in1=vv[:, :cn, D:].rearrange('p c (h o) -> p c h o', o=1)
                                .to_broadcast([128, cn, HEADS, DH]),
                            op=AO.mult)
                        for j in range(cn):
                            last = (p + j == nch - 1)
                            nc.tensor.matmul(out=wvz[:], lhsT=A2[:, j * 128:(j + 1) * 128],
                                             rhs=vse2[:, j * (D + HEADS):(j + 1) * (D + HEADS)],
                                             start=first, stop=last)
                            first = False
                        p += cn
                    at_off += nch
                    # epilogue for this subwindow
                    zr = sb3.tile([128, HEADS], f32, tag='zr')
                    nc.vector.tensor_scalar_add(out=zr[:], in0=wvz[:, D:],
                                                scalar1=6.7379470e-09)  # 1e-6 * exp(-5)
                    zrec = sb3.tile([128, HEADS], f32, tag='zrec')
                    nc.vector.reciprocal(out=zrec[:], in_=zr[:])
                    attn = sb3.tile([128, D], bf16, tag='attn')
                    nc.vector.tensor_tensor(
                        out=attn[:].rearrange('p (h e) -> p h e', e=DH),
                        in0=wvz[:, :D].rearrange('p (h e) -> p h e', e=DH),
                        in1=zrec[:].rearrange('p (h o) -> p h o', o=1).to_broadcast([128, HEADS, DH]),
                        op=AO.mult)
                    atr_ps = psE.tile([128, D], bf16, tag='atr')
                    nc.tensor.transpose(out=atr_ps[:], in_=attn[:], identity=ident[:])
                    atr = sb3.tile([128, D], bf16, tag='atrsb')
                    nc.vector.tensor_copy(out=atr[:], in_=atr_ps[:])
                    ho_ps = psE.tile([128, D], f32, tag='ho')
                    nc.tensor.matmul(out=ho_ps[:], lhsT=atr[:], rhs=ow_sb[:], start=True, stop=True)
                    # LN1 (fused): hob = ho+o_b with row-sum accumulated
                    hob = sb3.tile([128, D], f32, tag='hob')
                    mus = sb3.tile([128, 1], f32, tag='mus')
                    nc.vector.scalar_tensor_tensor(out=hob[:], in0=ho_ps[:], scalar=1.0,
                                                   op0=AO.mult, in1=ob_sb[:], op1=AO.add,
                                                   accum_out=mus[:])
                    mu = sb3.tile([128, 1], f32, tag='mu')
                    nc.vector.tensor_scalar_mul(out=mu[:], in0=mus[:], scalar1=1.0 / D)
                    xc = sb3.tile([128, D], f32, tag='xc')
                    nc.vector.scalar_tensor_tensor(out=xc[:], in0=hob[:], scalar=mu[:],
                                                   op0=AO.subtract, in1=ones_d[:], op1=AO.mult)
                    sq = sb3.tile([128, D], f32, tag='sq')
                    vr = sb3.tile([128, 1], f32, tag='vr')
                    nc.scalar.activation(out=sq[:], in_=xc[:], func=AF.Square,
                                         accum_out=vr[:])
                    nc.vector.tensor_scalar(out=vr[:], in0=vr[:], scalar1=1.0 / D,
                                            scalar2=1e-5, op0=AO.mult, op1=AO.add)
                    sd = sb3.tile([128, 1], f32, tag='sd')
                    nc.scalar.activation(out=sd[:], in_=vr[:], func=AF.Sqrt)
                    rsd = sb3.tile([128, 1], f32, tag='rsd')
                    nc.vector.reciprocal(out=rsd[:], in_=sd[:])
                    h1f = sb3.tile([128, D], f32, tag='h1f')
                    nc.vector.scalar_tensor_tensor(out=h1f[:], in0=xc[:], scalar=rsd[:],
                                                   op0=AO.mult, in1=csb['ln1_g'][:], op1=AO.mult)
                    if sw % 2 == 0:
                        h1p = h1_pool.tile([128, 2 * D], bf16, tag=f'h1p_{sw // 2}',
                                           name=f'h1p_{sw // 2}')
                        h1t.append(h1p)
                    nc.vector.tensor_tensor(out=h1t[-1][:, (sw % 2) * D:(sw % 2 + 1) * D],
                                            in0=h1f[:], in1=csb['ln1_b'][:], op=AO.add)

            nc.leave_named_scope('ph3_attn', sc3[0], False)
            # =========== phase 4: graphNorm2 stats + finalize ===========
            sc4 = nc.enter_named_scope('ph45_tail', False)
            with (
                tc.tile_pool(name='p4keep', bufs=1) as keep4,
                tc.tile_pool(name='p4sb', bufs=3) as sb4,
                tc.tile_pool(name='p4ps', bufs=1, space='PSUM') as ps4,
                tc.tile_pool(name='p4ps2', bufs=1, space='PSUM') as ps4b,
            ):
                sum2 = ps4.tile([GMAX, D], f32, tag='sum2')
                sq2 = ps4.tile([GMAX, D], f32, tag='sq2')
                NSP = NSW // 2
                for sp in range(NSP):
                    h1p = h1t[sp]
                    B2 = sb4.tile([128, 2 * GMAX], bf16, tag='B2')
                    nc.vector.tensor_tensor(
                        out=B2[:].rearrange('p (c g) -> p c g', g=GMAX),
                        in0=segs[:, 2 * sp:2 * sp + 2].rearrange('p (c o) -> p c o', o=1)
                            .to_broadcast([128, 2, GMAX]),
                        in1=iotaGG[:].rearrange('p (c g) -> p c g', g=GMAX),
                        op=AO.is_equal)
                    hsq = sb4.tile([128, 2 * D], bf16, tag='h2sq')
                    nc.gpsimd.tensor_tensor(out=hsq[:], in0=h1p[:], in1=h1p[:], op=AO.mult)
                    for j in range(2):
                        s = 2 * sp + j
                        nc.tensor.matmul(out=sum2[:], lhsT=B2[:, j * GMAX:(j + 1) * GMAX],
                                         rhs=h1p[:, j * D:(j + 1) * D],
                                         start=(s == 0), stop=(s == NSW - 1))
                        nc.tensor.matmul(out=sq2[:], lhsT=B2[:, j * GMAX:(j + 1) * GMAX],
                                         rhs=hsq[:, j * D:(j + 1) * D],
                                         start=(s == 0), stop=(s == NSW - 1))
                mean2 = keep4.tile([GMAX, D], f32)
                nc.vector.tensor_tensor(out=mean2[:], in0=sum2[:],
                                        in1=invc_sb[:].to_broadcast([GMAX, D]), op=AO.mult)
                ex22 = keep4.tile([GMAX, D], f32)
                nc.vector.tensor_tensor(out=ex22[:], in0=sq2[:],
                                        in1=invc_sb[:].to_broadcast([GMAX, D]), op=AO.mult)
                msq2 = keep4.tile([GMAX, D], f32)
                nc.vector.tensor_tensor(out=msq2[:], in0=mean2[:], in1=mean2[:], op=AO.mult)
                nc.vector.tensor_tensor(out=msq2[:], in0=msq2[:], in1=csb['gn2_msfac16'][:], op=AO.mult)
                var2 = keep4.tile([GMAX, D], f32)
                nc.vector.tensor_tensor(out=var2[:], in0=ex22[:], in1=msq2[:], op=AO.subtract)
                nc.vector.tensor_scalar_add(out=var2[:], in0=var2[:], scalar1=1e-6)
                std2 = keep4.tile([GMAX, D], f32)
                nc.scalar.activation(out=std2[:], in_=var2[:], func=AF.Sqrt)
                rstd2 = keep4.tile([GMAX, D], f32)
                nc.vector.reciprocal(out=rstd2[:], in_=std2[:])
                alpha2 = keep4.tile([GMAX, D], f32)
                nc.vector.tensor_tensor(out=alpha2[:], in0=rstd2[:], in1=csb['gn2_w16'][:], op=AO.mult)
                beta2 = keep4.tile([GMAX, D], f32)
                nc.vector.tensor_tensor(out=beta2[:], in0=mean2[:], in1=csb['gn2_ms16'][:], op=AO.mult)
                nc.vector.tensor_tensor(out=beta2[:], in0=beta2[:], in1=alpha2[:], op=AO.mult)
                nc.vector.tensor_tensor(out=beta2[:], in0=csb['gn2_b16'][:], in1=beta2[:], op=AO.subtract)
                alpha2h = keep4.tile([GMAX, D], bf16)
                nc.vector.tensor_copy(out=alpha2h[:], in_=alpha2[:])
                beta2h = keep4.tile([GMAX, D], bf16)
                nc.vector.tensor_copy(out=beta2h[:], in_=beta2[:])

                # =========== phase 5: gn2 apply + FFN + LN2 (sw pairs) ===========
                vr_all = keep4.tile([128, NSW], f32)
                xct = []
                for sp in range(NSP):
                    h1p = h1t[sp]
                    B2 = sb4.tile([128, 2 * GMAX], bf16, tag='B3')
                    nc.vector.tensor_tensor(
                        out=B2[:].rearrange('p (c g) -> p c g', g=GMAX),
                        in0=segs[:, 2 * sp:2 * sp + 2].rearrange('p (c o) -> p c o', o=1)
                            .to_broadcast([128, 2, GMAX]),
                        in1=iotaGG[:].rearrange('p (c g) -> p c g', g=GMAX),
                        op=AO.is_equal)
                    bt_ps = ps4b.tile([GMAX, 256], bf16, tag='bt2')
                    for j in range(2):
                        nc.tensor.transpose(out=bt_ps[:, j * 128:(j + 1) * 128],
                                            in_=B2[:, j * GMAX:(j + 1) * GMAX],
                                            identity=ident[:])
                    bt = sb4.tile([GMAX, 256], bf16, tag='bt2sb')
                    nc.scalar.activation(out=bt[:], in_=bt_ps[:], func=AF.Copy)
                    ab_ps = ps4b.tile([128, 4 * D], f32, tag='ab2')
                    for j in range(2):
                        nc.tensor.matmul(out=ab_ps[:, j * 2 * D:j * 2 * D + D],
                                         lhsT=bt[:, j * 128:(j + 1) * 128],
                                         rhs=alpha2h[:], start=True, stop=False)
                        nc.tensor.matmul(out=ab_ps[:, j * 2 * D + D:(j + 1) * 2 * D],
                                         lhsT=bt[:, j * 128:(j + 1) * 128],
                                         rhs=beta2h[:], start=True, stop=True)
                    ab_v = ab_ps[:].rearrange('p (c x d) -> p c x d', x=2, d=D)
                    h2 = sb4.tile([128, 2 * D], bf16, tag='h2')
                    h2v = h2[:].rearrange('p (c d) -> p c d', d=D)
                    h1v = h1p[:].rearrange('p (c d) -> p c d', d=D)
                    nc.vector.tensor_tensor(out=h2v, in0=h1v, in1=ab_v[:, :, 0, :], op=AO.mult)
                    nc.vector.tensor_tensor(out=h2v, in0=h2v, in1=ab_v[:, :, 1, :], op=AO.add)
                    h2t_ps = ps4b.tile([128, 2 * D], bf16, tag='h2t')
                    for j in range(2):
                        nc.tensor.transpose(out=h2t_ps[:, j * D:(j + 1) * D],
                                            in_=h2[:, j * D:(j + 1) * D], identity=ident[:])
                    h2tt = sb4.tile([128, 2 * D], bf16, tag='h2tsb')
                    nc.scalar.activation(out=h2tt[:], in_=h2t_ps[:], func=AF.Copy)
                    f1_ps = ps4b.tile([128, 4 * D], f32, tag='f1')
                    for j in range(2):
                        nc.tensor.matmul(out=f1_ps[:, j * 2 * D:(j + 1) * 2 * D],
                                         lhsT=h2tt[:, j * D:(j + 1) * D],
                                         rhs=ffn1_sb[:], start=True, stop=True)
                    fr = sb4.tile([128, 4 * D], bf16, tag='fr')
                    nc.vector.tensor_tensor(
                        out=fr[:].rearrange('p (c x) -> p c x', x=2 * D),
                        in0=f1_ps[:].rearrange('p (c x) -> p c x', x=2 * D),
                        in1=ffn1b_sb[:].rearrange('p (o x) -> p o x', o=1)
                            .to_broadcast([128, 2, 2 * D]),
                        op=AO.add)
                    nc.gpsimd.tensor_scalar_max(out=fr[:], in0=fr[:], scalar1=0.0)
                    frt_ps = ps4b.tile([128, 4 * D], bf16, tag='frt')
                    for j in range(4):
                        nc.tensor.transpose(out=frt_ps[:, j * D:(j + 1) * D],
                                            in_=fr[:, j * D:(j + 1) * D], identity=ident[:])
                    frt = sb4.tile([128, 4 * D], bf16, tag='frtsb')
                    nc.scalar.activation(out=frt[:], in_=frt_ps[:], func=AF.Copy)
                    h3_ps = ps4b.tile([128, 2 * D], f32, tag='h3')
                    for j in range(2):
                        nc.tensor.matmul(out=h3_ps[:, j * D:(j + 1) * D],
                                         lhsT=frt[:, j * 2 * D:j * 2 * D + D],
                                         rhs=ffn2_sb[:, :D], start=True, stop=False)
                        nc.tensor.matmul(out=h3_ps[:, j * D:(j + 1) * D],
                                         lhsT=frt[:, j * 2 * D + D:(j + 1) * 2 * D],
                                         rhs=ffn2_sb[:, D:], start=False, stop=True)
                    h3b = sb4.tile([128, 2 * D], f32, tag='h3b')
                    nc.vector.tensor_tensor(
                        out=h3b[:].rearrange('p (c d) -> p c d', d=D),
                        in0=h3_ps[:].rearrange('p (c d) -> p c d', d=D),
                        in1=ffn2b_sb[:].rearrange('p (o d) -> p o d', o=1)
                            .to_broadcast([128, 2, D]),
                        op=AO.add)
                    # LN2 stats (sqrt deferred and batched)
                    mu = sb4.tile([128, 2], f32, tag='mu2')
                    nc.vector.reduce_sum(out=mu[:].rearrange('p (c o) -> p c o', o=1),
                                         in_=h3b[:].rearrange('p (c d) -> p c d', d=D),
                                         axis=mybir.AxisListType.X)
                    nc.vector.tensor_scalar_mul(out=mu[:], in0=mu[:], scalar1=1.0 / D)
                    xc = keep4.tile([128, 2 * D], f32, tag=f'xc2_{sp}', name=f'xc2_{sp}')
                    nc.vector.tensor_tensor(
                        out=xc[:].rearrange('p (c d) -> p c d', d=D),
                        in0=h3b[:].rearrange('p (c d) -> p c d', d=D),
                        in1=mu[:].rearrange('p (c o) -> p c o', o=1).to_broadcast([128, 2, D]),
                        op=AO.subtract)
                    sq = sb4.tile([128, 2 * D], f32, tag='sq2w')
                    nc.gpsimd.tensor_tensor(out=sq[:], in0=xc[:], in1=xc[:], op=AO.mult)
                    nc.vector.reduce_sum(
                        out=vr_all[:, 2 * sp:2 * sp + 2].rearrange('p (c o) -> p c o', o=1),
                        in_=sq[:].rearrange('p (c d) -> p c d', d=D),
                        axis=mybir.AxisListType.X)
                    xct.append(xc)
                # batched 1/sqrt(var/D + eps) for all subwindows
                nc.vector.tensor_scalar(out=vr_all[:], in0=vr_all[:], scalar1=1.0 / D,
                                        scalar2=1e-5, op0=AO.mult, op1=AO.add)
                sd_all = keep4.tile([128, NSW], f32)
                nc.scalar.activation(out=sd_all[:], in_=vr_all[:], func=AF.Sqrt)
                rsd_all = keep4.tile([128, NSW], f32)
                nc.vector.reciprocal(out=rsd_all[:], in_=sd_all[:])
                for sp in range(NSP):
                    ov = sb4.tile([128, 2 * D], f32, tag='ov')
                    ovv = ov[:].rearrange('p (c d) -> p c d', d=D)
                    nc.vector.tensor_tensor(
                        out=ovv, in0=xct[sp][:].rearrange('p (c d) -> p c d', d=D),
                        in1=rsd_all[:, 2 * sp:2 * sp + 2].rearrange('p (c o) -> p c o', o=1)
                            .to_broadcast([128, 2, D]),
                        op=AO.mult)
                    nc.gpsimd.tensor_tensor(
                        out=ovv, in0=ovv,
                        in1=csb['ln2_g'][:].rearrange('p (o d) -> p o d', o=1)
                            .to_broadcast([128, 2, D]),
                        op=AO.mult)
                    nc.gpsimd.tensor_tensor(
                        out=ovv, in0=ovv,
                        in1=csb['ln2_b'][:].rearrange('p (o d) -> p o d', o=1)
                            .to_broadcast([128, 2, D]),
                        op=AO.add)
                    nc.sync.dma_start(out=out_sl[2 * sp * 128:(2 * sp + 1) * 128, :],
                                      in_=ov[:, :D])
                    nc.sync.dma_start(out=out_sl[(2 * sp + 1) * 128:(2 * sp + 2) * 128, :],
                                      in_=ov[:, D:])
            nc.leave_named_scope('ph45_tail', sc4[0], False)
            h1_cm.__exit__(None, None, None)
            if debug:
                nc.sync.dma_start(out=hn_dbg[:], in_=hn_local[:])
                nc.sync.dma_start(out=q_dbg[:], in_=q_local[:])
                nc.sync.dma_start(out=kv_dbg[:], in_=kv_local[:])

    nc.finalize()
    return nc


def kernel(**inputs) -> np.ndarray:
    _ensure_hooks()
    from concourse.bass_utils import run_bass_kernel_spmd

    static, in_maps, meta = preprocess(inputs)
    key = tuple(sorted((k, v) for k, v in static.items()))
    if key not in _PROGRAM_CACHE:
        _PROGRAM_CACHE[key] = build_program(static)
    nc = _PROGRAM_CACHE[key]

    trace = os.environ.get("KERNEL_TRACE") == "1"
    res = run_bass_kernel_spmd(nc, in_maps, list(range(NCORES)), trace=trace)
    if trace and res.exec_time_ns:
        print("HW exec time:", res.exec_time_ns, "ns")
    out = np.zeros((N_NODES, D), np.float32)
    for c in range(NCORES):
        n0, n1 = int(meta['n0'][c]), int(meta['n1'][c])
        out[n0:n1] = res.results[c]['out_slice'][:n1 - n0]
    return out


# revision 50
# speedup vs baseline: 1.1824x; 1.1824x over previous
"""GTLayer (relational graph transformer layer) on 8 Trainium2 NeuronCores.

Strategy
--------
Nodes are partitioned across 8 cores in graph-aligned contiguous slices
(graphNorm stays core-local). Edges live with the core that owns dst.
Per core, dst nodes are processed in 256-node windows.

- graphNorm1: slice-local stats (one-pass sum/sumsq via one-hot matmuls),
  normalize, then AllGather hn -> global gather table (bf16).
- RelConv (Q|K|V fused, 384 cols): edges sorted by (window, src-half,
  relation), each (w,half,r) run padded to 128-slot chunks (>=1 pad slot per
  chunk, index 0, key -1). hn[src] rows fetched with the ext-isa dma_gather
  (int16 indices, signed, two base offsets cover the global table).
  Aggregation is one-hot matmuls in bf16 (PSUM accumulates f32).
- Attention: same chunk machinery per (window, half), all matmuls bf16.
- Epilogue per 128 rows: attn = wV/(z+eps), hO = attn@o_w+o_b, LN1,
  graphNorm2 (stats pass over SBUF-resident h1), FFN, LN2 -> output slice.

All per-core variation is in input data (indices/keys); the SPMD program is
identical across cores (chunk counts are max'ed over cores).
"""
import os
import sys
import types
import numpy as np
BF16 = np.float16

NCORES = 8
N_NODES = 100000
N_EDGES = 600000
D = 128
REL = 9
NG = 64
HEADS = 8
DH = 16
WIN = 256          # dst window (2 x 128 subwindows)
GMAX = 16          # max graphs per core
CALL_MAX = 8       # max chunks (of 128 slots) per dma_gather call


def _ensure_hooks():
    if "antenv.axon_hooks" not in sys.modules:
        hooks = types.ModuleType("antenv.axon_hooks")
        h = [None]
        hooks.set_axon_ntff_profile_hook = lambda v: h.__setitem__(0, v)
        hooks.get_axon_ntff_profile_hook = lambda: h[0]
        sys.modules["antenv.axon_hooks"] = hooks
        try:
            from trn_agent_boot.trn_boot import _ntff_profile_via_ctypes
            hooks.set_axon_ntff_profile_hook(
                _ntff_profile_via_ctypes("/opt/axon/libaxon_pjrt.so"))
        except Exception:
            pass


# ----------------------------------------------------------------------------
# Host preprocessing
# ----------------------------------------------------------------------------

def _pack_idx16(idx):
    """int16 index array -> [128, n/16] wrapped+replicated layout."""
    n = len(idx)
    assert n % 16 == 0
    blk = idx.reshape(n // 16, 16).T
    return np.tile(blk, (8, 1)).astype(np.int16)


def _layout_slots(order_edges, idx_vals, key_vals, n_chunks):
    """Place edges into n_chunks*128 slots, <=127 real per chunk, last slot of
    each chunk is a pad (idx 0, key -1). Returns (idx int32, key int16)."""
    tot = n_chunks * 128
    idx = np.zeros(tot, np.int32)
    key = np.full(tot, -1, np.int16)
    ne = len(order_edges)
    pos = 0
    ei = 0
    for c in range(n_chunks):
        take = min(127, ne - ei)
        if take > 0:
            sl = slice(c * 128, c * 128 + take)
            idx[sl] = idx_vals[order_edges[ei:ei + take]]
            key[sl] = key_vals[order_edges[ei:ei + take]]
            ei += take
    assert ei == ne, (ei, ne, n_chunks)
    return idx, key


def preprocess(inputs):
    h = np.asarray(inputs['h'], np.float32)
    src = np.asarray(inputs['src']).astype(np.int64)
    dst = np.asarray(inputs['dst']).astype(np.int64)
    et = np.asarray(inputs['etypes']).astype(np.int64)
    seg = np.asarray(inputs['seg']).astype(np.int64)

    # --- graph-aligned node partition ---
    gstart = np.searchsorted(seg, np.arange(NG + 1))  # graph g: [gstart[g], gstart[g+1])
    bounds = [0]
    for c in range(1, NCORES):
        target = c * N_NODES / NCORES
        g = int(np.argmin(np.abs(gstart - target)))
        bounds.append(int(gstart[g]))
    bounds.append(N_NODES)
    n0 = np.array(bounds[:-1]); n1 = np.array(bounds[1:])
    sizes = n1 - n0
    P_NODES = int(np.ceil(sizes.max() / WIN) * WIN)
    NW = P_NODES // WIN
    # lo/hi half-tables (each AllGathered separately so the lo collective can
    # overlap with compute of the hi half). int16 gather reach: rows/2 <= 32768.
    M_LO = (NW // 2) * WIN
    M_HI = P_NODES - M_LO
    ROWS_LO = NCORES * M_LO
    ROWS_HI = NCORES * M_HI
    assert ROWS_LO // 2 <= 32768 and ROWS_HI // 2 <= 32768

    owner = np.searchsorted(n1, np.arange(N_NODES), side='right')
    off_all = np.arange(N_NODES) - n0[owner]

    # --- per-core graph info ---
    g0 = np.searchsorted(gstart, n0, side='right') - 1  # first graph on core
    counts_g = np.diff(gstart).astype(np.float32)

    owner_s = owner[src]
    off_s = off_all[src]
    half = (off_s >= M_LO).astype(np.int64)
    ecore = owner[dst]
    dst_off = dst - n0[ecore]
    w_e = dst_off // WIN
    dl_e = (dst_off % WIN).astype(np.float32)
    idx_rel = np.where(half == 0,
                       owner_s * M_LO + off_s - ROWS_LO // 2,
                       owner_s * M_HI + (off_s - M_LO) - ROWS_HI // 2).astype(np.int32)

    # --- relconv structure: runs (w, half); slots sorted by key=et*256+dl so
    # chunks span relation boundaries (one matmul per (chunk, rel-present)) ---
    ckey = (et * 256 + dst_off % WIN).astype(np.int16)
    rkey = w_e * 2 + half
    rc_counts = np.zeros((NCORES, NW * 2), np.int64)
    for c in range(NCORES):
        m = ecore == c
        rc_counts[c] = np.bincount(rkey[m], minlength=NW * 2)
    rc_chunks = np.ceil(rc_counts / 127.0).max(0).astype(np.int64)

    # --- attention structure: segments (sw128, half) ---
    sw_e = dst_off // 128
    dl128 = (dst_off % 128).astype(np.int16)
    NSW = NW * 2
    akey = sw_e * 2 + half
    at_counts = np.zeros((NCORES, NSW * 2), np.int64)
    for c in range(NCORES):
        m = ecore == c
        at_counts[c] = np.bincount(akey[m], minlength=NSW * 2)
    at_chunks = np.maximum(np.ceil(at_counts / 127.0).max(0), 1).astype(np.int64)

    RC_CHUNKS = int(rc_chunks.sum())
    AT_CHUNKS = int(at_chunks.sum())

    # --- per-core data arrays ---
    # rc_sched[chunk] = tuple of relations present in that chunk on ANY core
    rc_rels = [set() for _ in range(RC_CHUNKS)]
    in_maps = []
    for c in range(NCORES):
        m = np.nonzero(ecore == c)[0]
        rk = rkey[m]
        order = np.argsort(rk * 4096 + ckey[m].astype(np.int64), kind='stable')
        edges_sorted = m[order]
        rk_sorted = rk[order]
        run_start = np.searchsorted(rk_sorted, np.arange(NW * 2))
        run_end = np.searchsorted(rk_sorted, np.arange(NW * 2) + 1)

        rc_idx = np.zeros(RC_CHUNKS * 128, np.int32)
        rc_key = np.full(RC_CHUNKS * 128, -1, np.int16)
        coff = 0
        for q in range(NW * 2):
            nch = int(rc_chunks[q])
            eidx = edges_sorted[run_start[q]:run_end[q]]
            ii, kk = _layout_slots(eidx, idx_rel, ckey, nch)
            rc_idx[coff * 128:(coff + nch) * 128] = ii
            rc_key[coff * 128:(coff + nch) * 128] = kk
            for k in range(nch):
                ee = eidx[k * 127:(k + 1) * 127]
                rc_rels[coff + k].update(int(x) for x in np.unique(et[ee]))
            coff += nch
        assert coff == RC_CHUNKS

        ak = akey[m]
        aorder = np.argsort(ak, kind='stable')
        aedges = m[aorder]
        ak_sorted = ak[aorder]
        astart = np.searchsorted(ak_sorted, np.arange(NSW * 2))
        aend = np.searchsorted(ak_sorted, np.arange(NSW * 2) + 1)
        at_idx = np.zeros(AT_CHUNKS * 128, np.int32)
        at_key = np.full(AT_CHUNKS * 128, -1, np.int16)
        coff = 0
        for q in range(NSW * 2):
            nch = int(at_chunks[q])
            eidx = aedges[astart[q]:aend[q]]
            ii, kk = _layout_slots(eidx, idx_rel, dl128, nch)
            at_idx[coff * 128:(coff + nch) * 128] = ii
            at_key[coff * 128:(coff + nch) * 128] = kk
            coff += nch
        assert coff == AT_CHUNKS

        hs = np.zeros((P_NODES, D), np.float32)
        hs[:sizes[c]] = h[n0[c]:n1[c]]
        segl = np.full(P_NODES, -1.0, np.float32)
        segl[:sizes[c]] = (seg[n0[c]:n1[c]] - g0[c]).astype(np.float32)
        ginc = np.zeros((GMAX, 1), np.float32)
        ng_c = int(seg[n1[c] - 1] - g0[c]) + 1
        assert ng_c <= GMAX
        ginc[:ng_c, 0] = 1.0 / counts_g[g0[c]:g0[c] + ng_c]

        im = {
            'h_slice': hs.astype(BF16),
            'seg_col': segl.reshape(NW * 2, 128).T.copy(),   # [128, NW*2]
            'inv_cnt': ginc,
            'rc_idx': _pack_idx16(rc_idx.astype(np.int16)),
            'rc_key': rc_key.reshape(RC_CHUNKS, 128).T.copy(),  # [128, RC_CHUNKS]
            'at_idx': _pack_idx16(at_idx.astype(np.int16)),
            'at_key': at_key.reshape(AT_CHUNKS, 128).T.copy(),
        }
        in_maps.append(im)

    # --- shared weights (same for all cores) ---
    def A(x):
        return np.ascontiguousarray(np.asarray(x, np.float32))
    Wrel = np.concatenate([
        np.einsum('rb,bio->rio', A(inputs[f'{nm}_coeff']), A(inputs[f'{nm}_basis']))
        for nm in ('q', 'k', 'v')], axis=2)            # [9, 128, 384]
    w_shared = {
        'w_rel': A(Wrel.reshape(REL * D, 3 * D)).astype(BF16),
        'w_loop': np.concatenate([A(inputs[f'{nm}_loop']) for nm in ('q', 'k', 'v')], 1).astype(BF16),
        'b_qkv': np.tile(np.concatenate([A(inputs[f'{nm}_bias']) for nm in ('q', 'k', 'v')])[None, :], (128, 1)),
        'o_w': A(inputs['o_w']).astype(BF16), 'o_b': np.tile(A(inputs['o_b'])[None, :], (128, 1)),
        'ffn1': A(inputs['ffn1_w']).astype(BF16), 'ffn1_b': np.tile(A(inputs['ffn1_b'])[None, :], (128, 1)),
        'ffn2': A(inputs['ffn2_w']).astype(BF16), 'ffn2_b': np.tile(A(inputs['ffn2_b'])[None, :], (128, 1)),
        'ln1_g': np.tile(A(inputs['ln1_g'])[None, :], (128, 1)),
        'ln1_b': np.tile(A(inputs['ln1_b'])[None, :], (128, 1)),
        'ln2_g': np.tile(A(inputs['ln2_g'])[None, :], (128, 1)),
        'ln2_b': np.tile(A(inputs['ln2_b'])[None, :], (128, 1)),
    }
    for nm in ('gn1', 'gn2'):
        w = A(inputs[f'{nm}_w']); b = A(inputs[f'{nm}_b']); ms = A(inputs[f'{nm}_ms'])
        w_shared[f'{nm}_w16'] = np.tile(w[None, :], (GMAX, 1))
        w_shared[f'{nm}_b16'] = np.tile(b[None, :], (GMAX, 1))
        w_shared[f'{nm}_ms16'] = np.tile(ms[None, :], (GMAX, 1))
        w_shared[f'{nm}_msfac16'] = np.tile((ms * (2 - ms))[None, :], (GMAX, 1))
    for im in in_maps:
        im.update(w_shared)

    static = dict(P_NODES=P_NODES, NW=NW, M_LO=M_LO, M_HI=M_HI,
                  rc_chunks=tuple(int(x) for x in rc_chunks),
                  rc_sched=tuple(tuple(sorted(s)) for s in rc_rels),
                  at_chunks=tuple(int(x) for x in at_chunks),
                  RC_CHUNKS=RC_CHUNKS, AT_CHUNKS=AT_CHUNKS)
    meta = dict(n0=n0, n1=n1, sizes=sizes)
    return static, in_maps, meta


# ----------------------------------------------------------------------------
# Bass program
# ----------------------------------------------------------------------------

_PROGRAM_CACHE = {}


def build_program(st):
    import concourse.bass as bass
    import concourse.bacc as bacc
    import concourse.mybir as mybir
    import concourse.tile as tile
    from concourse.tile import TileContext
    from concourse.masks import make_identity
    from bass_rust import add_dep_helper

    P_NODES = st['P_NODES']; NW = st['NW']
    M_LO = st['M_LO']; M_HI = st['M_HI']
    ROWS_LO = NCORES * M_LO; ROWS_HI = NCORES * M_HI
    rc_chunks = st['rc_chunks']; at_chunks = st['at_chunks']
    rc_sched = st['rc_sched']
    RC_CHUNKS = st['RC_CHUNKS']; AT_CHUNKS = st['AT_CHUNKS']
    f32 = mybir.dt.float32
    bf16 = mybir.dt.float16
    i16 = mybir.dt.int16
    AO = mybir.AluOpType
    AF = mybir.ActivationFunctionType

    nc = bacc.Bacc()

    # --- I/O ---
    h_slice = nc.declare_dram_parameter('h_slice', [P_NODES, D], bf16, isOutput=False)
    seg_col = nc.declare_dram_parameter('seg_col', [128, NW * 2], f32, isOutput=False)
    inv_cnt = nc.declare_dram_parameter('inv_cnt', [GMAX, 1], f32, isOutput=False)
    rc_idx = nc.declare_dram_parameter('rc_idx', [128, RC_CHUNKS * 8], i16, isOutput=False)
    rc_keyd = nc.declare_dram_parameter('rc_key', [128, RC_CHUNKS], i16, isOutput=False)
    at_idx = nc.declare_dram_parameter('at_idx', [128, AT_CHUNKS * 8], i16, isOutput=False)
    at_keyd = nc.declare_dram_parameter('at_key', [128, AT_CHUNKS], i16, isOutput=False)
    w_rel = nc.declare_dram_parameter('w_rel', [REL * D, 3 * D], bf16, isOutput=False)
    w_loop = nc.declare_dram_parameter('w_loop', [D, 3 * D], bf16, isOutput=False)
    b_qkv = nc.declare_dram_parameter('b_qkv', [128, 3 * D], f32, isOutput=False)
    o_w = nc.declare_dram_parameter('o_w', [D, D], bf16, isOutput=False)
    o_b = nc.declare_dram_parameter('o_b', [128, D], f32, isOutput=False)
    ffn1 = nc.declare_dram_parameter('ffn1', [D, 2 * D], bf16, isOutput=False)
    ffn1_b = nc.declare_dram_parameter('ffn1_b', [128, 2 * D], f32, isOutput=False)
    ffn2 = nc.declare_dram_parameter('ffn2', [2 * D, D], bf16, isOutput=False)
    ffn2_b = nc.declare_dram_parameter('ffn2_b', [128, D], f32, isOutput=False)
    cdecl = {}
    for nm in ('ln1_g', 'ln1_b', 'ln2_g', 'ln2_b'):
        cdecl[nm] = nc.declare_dram_parameter(nm, [128, D], f32, isOutput=False)
    for nm in ('gn1', 'gn2'):
        for sfx in ('w16', 'b16', 'ms16', 'msfac16'):
            cdecl[f'{nm}_{sfx}'] = nc.declare_dram_parameter(
                f'{nm}_{sfx}', [GMAX, D], f32, isOutput=False)
    out_sl = nc.declare_dram_parameter('out_slice', [P_NODES, D], f32, isOutput=True)

    # --- internal DRAM ---
    hn_local = nc.dram_tensor('hn_local', [P_NODES, D], bf16)
    q_local = nc.dram_tensor('q_local', [P_NODES, D], bf16)
    kv_local = nc.dram_tensor('kv_local', [P_NODES, 2 * D], bf16)
    debug = os.environ.get('KERNEL_DEBUG') == '1'
    if debug:
        hn_dbg = nc.declare_dram_parameter('hn_dbg', [P_NODES, D], bf16, isOutput=True)
        q_dbg = nc.declare_dram_parameter('q_dbg', [P_NODES, D], bf16, isOutput=True)
        kv_dbg = nc.declare_dram_parameter('kv_dbg', [P_NODES, 2 * D], bf16, isOutput=True)
    hn_lo = nc.dram_tensor('hn_lo', [NCORES, M_LO, D], bf16, addr_space='Shared')
    hn_hi = nc.dram_tensor('hn_hi', [NCORES, M_HI, D], bf16, addr_space='Shared')
    kv_lo = nc.dram_tensor('kv_lo', [NCORES, M_LO, 2 * D], bf16, addr_space='Shared')
    kv_hi = nc.dram_tensor('kv_hi', [NCORES, M_HI, 2 * D], bf16, addr_space='Shared')
    # gather base views: idx 0 points at the middle row of each half-table
    hn_tab = [hn_lo[:].rearrange('c p d -> (c p) d')[ROWS_LO // 2:, :],
              hn_hi[:].rearrange('c p d -> (c p) d')[ROWS_HI // 2:, :]]
    kv_tab = [kv_lo[:].rearrange('c p d -> (c p) d')[ROWS_LO // 2:, :],
              kv_hi[:].rearrange('c p d -> (c p) d')[ROWS_HI // 2:, :]]

    with TileContext(nc) as tc:
        with tc.tile_pool(name='const', bufs=1) as cpool:
            # constants
            iota2304 = cpool.tile([128, REL * WIN], i16)
            nc.gpsimd.iota(iota2304[:], pattern=[[1, REL * WIN]], base=0,
                           channel_multiplier=0)
            iotaG = cpool.tile([128, GMAX], f32)
            nc.gpsimd.iota(iotaG[:], pattern=[[1, GMAX]], base=0,
                           channel_multiplier=0, allow_small_or_imprecise_dtypes=True)
            iotaGG = cpool.tile([128, 2 * GMAX], f32)
            nc.gpsimd.iota(iotaGG[:].rearrange('p (c f) -> p c f', f=GMAX),
                           pattern=[[0, 2], [1, GMAX]], base=0,
                           channel_multiplier=0, allow_small_or_imprecise_dtypes=True)
            iotaG4 = cpool.tile([128, 4 * GMAX], f32)
            nc.gpsimd.iota(iotaG4[:].rearrange('p (c f) -> p c f', f=GMAX),
                           pattern=[[0, 4], [1, GMAX]], base=0,
                           channel_multiplier=0, allow_small_or_imprecise_dtypes=True)
            ident = cpool.tile([128, 128], bf16)
            make_identity(nc, ident[:])
            ones1 = cpool.tile([1, 128], f32)
            nc.gpsimd.memset(ones1[:], 1.0)
            neg5 = cpool.tile([128, 1], f32)
            nc.gpsimd.memset(neg5[:], -5.0)
            epsz = cpool.tile([128, 1], f32)
            nc.gpsimd.memset(epsz[:], 6.7379470e-09)   # 1e-6 * exp(-5)
            zeros2304 = cpool.tile([128, REL * WIN], f32)
            nc.gpsimd.memset(zeros2304[:], 0.0)
            eps5 = cpool.tile([128, 1], f32)
            nc.gpsimd.memset(eps5[:], 1e-5)
            invd = cpool.tile([128, 1], f32)
            nc.gpsimd.memset(invd[:], 1.0 / D)
            ones_d = cpool.tile([128, D], f32)
            nc.gpsimd.memset(ones_d[:], 1.0)
            iota256 = cpool.tile([128, 256], i16)       # j % 128 pattern
            nc.gpsimd.iota(iota256[:].rearrange('p (c f) -> p c f', f=128),
                           pattern=[[0, 2], [1, 128]], base=0,
                           channel_multiplier=0)
            c40h = cpool.tile([128, 1], bf16)
            nc.gpsimd.memset(c40h[:], 40.0)

            segs = cpool.tile([128, NW * 2], f32)
            nc.sync.dma_start(out=segs[:], in_=seg_col[:])
            rck = cpool.tile([128, RC_CHUNKS], i16)
            nc.sync.dma_start(out=rck[:], in_=rc_keyd[:])
            atk = cpool.tile([128, AT_CHUNKS], i16)
            nc.sync.dma_start(out=atk[:], in_=at_keyd[:])
            rci = cpool.tile([128, RC_CHUNKS * 8], i16)
            nc.sync.dma_start(out=rci[:], in_=rc_idx[:])
            ati = cpool.tile([128, AT_CHUNKS * 8], i16)
            nc.sync.dma_start(out=ati[:], in_=at_idx[:])

            wrel_sb = cpool.tile([128, REL * 3 * D], bf16)  # r-th block at [:, r*384:(r+1)*384]
            for r in range(REL):
                nc.sync.dma_start(out=wrel_sb[:, r * 3 * D:(r + 1) * 3 * D],
                                  in_=w_rel[r * D:(r + 1) * D, :])
            wloop_sb = cpool.tile([128, 3 * D], bf16)
            nc.sync.dma_start(out=wloop_sb[:], in_=w_loop[:])
            bqkv_sb = cpool.tile([128, 3 * D], f32)
            nc.sync.dma_start(out=bqkv_sb[:], in_=b_qkv[:])
            ow_sb = cpool.tile([D, D], bf16)
            nc.sync.dma_start(out=ow_sb[:], in_=o_w[:])
            ob_sb = cpool.tile([128, D], f32)
            nc.sync.dma_start(out=ob_sb[:], in_=o_b[:])
            ffn1_sb = cpool.tile([D, 2 * D], bf16)
            nc.sync.dma_start(out=ffn1_sb[:], in_=ffn1[:])
            ffn1b_sb = cpool.tile([128, 2 * D], f32)
            nc.sync.dma_start(out=ffn1b_sb[:], in_=ffn1_b[:])
            ffn2_sb = cpool.tile([128, 2 * D], bf16)  # two K-chunks side by side
            nc.sync.dma_start(out=ffn2_sb[:, :D], in_=ffn2[:D, :])
            nc.sync.dma_start(out=ffn2_sb[:, D:], in_=ffn2[D:, :])
            ffn2b_sb = cpool.tile([128, D], f32)
            nc.sync.dma_start(out=ffn2b_sb[:], in_=ffn2_b[:])
            csb = {}
            for nm, dd in cdecl.items():
                t = cpool.tile(list(dd.shape), f32, tag=f'c_{nm}')
                nc.sync.dma_start(out=t[:], in_=dd[:])
                csb[nm] = t
            invc_sb = cpool.tile([GMAX, 1], f32)
            nc.sync.dma_start(out=invc_sb[:], in_=inv_cnt[:])

            NSW = NW * 2  # number of 128-row subwindows
            # subwindow groups of 4 (trailing 2 if NSW % 4): batch tail DVE ops
            groups = [4] * (NSW // 4)
            if NSW % 4:
                groups.append(NSW % 4)
            sw2grp = {}
            s0 = 0
            for gi, gsz in enumerate(groups):
                for j in range(gsz):
                    sw2grp[s0 + j] = (gi, j)
                s0 += gsz

            # =========== phase 1: graphNorm1 ===========
            sc1 = nc.enter_named_scope('ph1_gn1', False)
            with (
                tc.tile_pool(name='p1keep', bufs=1) as keep1,
                tc.tile_pool(name='p1sb', bufs=3) as sb1,
                tc.tile_pool(name='p1ps', bufs=1, space='PSUM') as ps1,
                tc.tile_pool(name='p1ps2', bufs=1, space='PSUM') as ps1b,
            ):
                sum_ps = ps1.tile([GMAX, D], f32, tag='sums')
                sq_ps = ps1.tile([GMAX, D], f32, tag='sqs')
                hwins = []
                for sp in range(NSW // 2):
                    hw = keep1.tile([128, 2 * D], bf16, tag=f'h_{sp}')
                    nc.sync.dma_start(out=hw[:, :D],
                                      in_=h_slice[2 * sp * 128:(2 * sp + 1) * 128, :])
                    nc.sync.dma_start(out=hw[:, D:],
                                      in_=h_slice[(2 * sp + 1) * 128:(2 * sp + 2) * 128, :])
                    B2 = sb1.tile([128, 2 * GMAX], bf16, tag='B1')
                    nc.vector.tensor_tensor(
                        out=B2[:].rearrange('p (c g) -> p c g', g=GMAX),
                        in0=segs[:, 2 * sp:2 * sp + 2].rearrange('p (c o) -> p c o', o=1)
                            .to_broadcast([128, 2, GMAX]),
                        in1=iotaGG[:].rearrange('p (c g) -> p c g', g=GMAX),
                        op=AO.is_equal)
                    hsq = sb1.tile([128, 2 * D], bf16, tag='hsq')
                    nc.vector.tensor_tensor(out=hsq[:], in0=hw[:], in1=hw[:], op=AO.mult)
                    for j in range(2):
                        s = 2 * sp + j
                        nc.tensor.matmul(out=sum_ps[:], lhsT=B2[:, j * GMAX:(j + 1) * GMAX],
                                         rhs=hw[:, j * D:(j + 1) * D],
                                         start=(s == 0), stop=(s == NSW - 1))
                        nc.tensor.matmul(out=sq_ps[:], lhsT=B2[:, j * GMAX:(j + 1) * GMAX],
                                         rhs=hsq[:, j * D:(j + 1) * D],
                                         start=(s == 0), stop=(s == NSW - 1))
                    hwins.append(hw)
                # finalize: alpha/beta [GMAX, D]
                mean = keep1.tile([GMAX, D], f32)
                nc.vector.tensor_tensor(out=mean[:], in0=sum_ps[:],
                                        in1=invc_sb[:].to_broadcast([GMAX, D]), op=AO.mult)
                ex2 = keep1.tile([GMAX, D], f32)
                nc.vector.tensor_tensor(out=ex2[:], in0=sq_ps[:],
                                        in1=invc_sb[:].to_broadcast([GMAX, D]), op=AO.mult)
                msq = keep1.tile([GMAX, D], f32)
                nc.vector.tensor_tensor(out=msq[:], in0=mean[:], in1=mean[:], op=AO.mult)
                nc.vector.tensor_tensor(out=msq[:], in0=msq[:], in1=csb['gn1_msfac16'][:], op=AO.mult)
                var = keep1.tile([GMAX, D], f32)
                nc.vector.tensor_tensor(out=var[:], in0=ex2[:], in1=msq[:], op=AO.subtract)
                nc.vector.tensor_scalar_add(out=var[:], in0=var[:], scalar1=1e-6)
                std = keep1.tile([GMAX, D], f32)
                nc.scalar.activation(out=std[:], in_=var[:], func=AF.Sqrt)
                rstd = keep1.tile([GMAX, D], f32)
                nc.vector.reciprocal(out=rstd[:], in_=std[:])
                alpha1 = keep1.tile([GMAX, D], f32)
                nc.vector.tensor_tensor(out=alpha1[:], in0=rstd[:], in1=csb['gn1_w16'][:], op=AO.mult)
                beta1 = keep1.tile([GMAX, D], f32)
                nc.vector.tensor_tensor(out=beta1[:], in0=mean[:], in1=csb['gn1_ms16'][:], op=AO.mult)
                nc.vector.tensor_tensor(out=beta1[:], in0=beta1[:], in1=alpha1[:], op=AO.mult)
                nc.vector.tensor_tensor(out=beta1[:], in0=csb['gn1_b16'][:], in1=beta1[:], op=AO.subtract)
                alpha1h = keep1.tile([GMAX, D], bf16)
                nc.vector.tensor_copy(out=alpha1h[:], in_=alpha1[:])
                beta1h = keep1.tile([GMAX, D], bf16)
                nc.vector.tensor_copy(out=beta1h[:], in_=beta1[:])
                # apply
                hn_stores = []
                SW_LO = M_LO // 128
                cc1 = [None, None]
                for sp in range(NSW // 2):
                    B2 = sb1.tile([128, 2 * GMAX], bf16, tag='B1b')
                    nc.vector.tensor_tensor(
                        out=B2[:].rearrange('p (c g) -> p c g', g=GMAX),
                        in0=segs[:, 2 * sp:2 * sp + 2].rearrange('p (c o) -> p c o', o=1)
                            .to_broadcast([128, 2, GMAX]),
                        in1=iotaGG[:].rearrange('p (c g) -> p c g', g=GMAX),
                        op=AO.is_equal)
                    bt_ps = ps1b.tile([GMAX, 256], bf16, tag='bt')
                    for j in range(2):
                        nc.tensor.transpose(out=bt_ps[:, j * 128:(j + 1) * 128],
                                            in_=B2[:, j * GMAX:(j + 1) * GMAX],
                                            identity=ident[:])
                    bt = sb1.tile([GMAX, 256], bf16, tag='btsb')
                    nc.scalar.activation(out=bt[:], in_=bt_ps[:], func=AF.Copy)
                    ab_ps = ps1b.tile([128, 4 * D], f32, tag='ab')
                    for j in range(2):
                        nc.tensor.matmul(out=ab_ps[:, j * 2 * D:j * 2 * D + D],
                                         lhsT=bt[:, j * 128:(j + 1) * 128],
                                         rhs=alpha1h[:], start=True, stop=False)
                        nc.tensor.matmul(out=ab_ps[:, j * 2 * D + D:(j + 1) * 2 * D],
                                         lhsT=bt[:, j * 128:(j + 1) * 128],
                                         rhs=beta1h[:], start=True, stop=True)
                    ab_v = ab_ps[:].rearrange('p (c x d) -> p c x d', x=2, d=D)
                    hnw = sb1.tile([128, 2 * D], bf16, tag='hnw')
                    hnv = hnw[:].rearrange('p (c d) -> p c d', d=D)
                    nc.vector.tensor_tensor(out=hnv,
                                            in0=hwins[sp][:].rearrange('p (c d) -> p c d', d=D),
                                            in1=ab_v[:, :, 0, :], op=AO.mult)
                    nc.vector.tensor_tensor(out=hnv, in0=hnv, in1=ab_v[:, :, 1, :], op=AO.add)
                    si0 = nc.sync.dma_start(out=hn_local[2 * sp * 128:(2 * sp + 1) * 128, :],
                                            in_=hnw[:, :D])
                    si1 = nc.sync.dma_start(out=hn_local[(2 * sp + 1) * 128:(2 * sp + 2) * 128, :],
                                            in_=hnw[:, D:])
                    hn_stores.extend([si0, si1])
                    if 2 * sp + 2 == SW_LO:
                        cc1[0] = nc.gpsimd.collective_compute(
                            'AllGather', AO.bypass, replica_groups=[list(range(NCORES))],
                            ins=[hn_local[0:M_LO, :]], outs=[hn_lo[:]])
                        for st_i in hn_stores:
                            add_dep_helper(cc1[0].ins, st_i.ins, True, 'cc1a after lo stores')
                cc1[1] = nc.gpsimd.collective_compute(
                    'AllGather', AO.bypass, replica_groups=[list(range(NCORES))],
                    ins=[hn_local[M_LO:, :]], outs=[hn_hi[:]])
                for st_i in hn_stores[SW_LO:]:
                    add_dep_helper(cc1[1].ins, st_i.ins, True, 'cc1b after hi stores')

            nc.leave_named_scope('ph1_gn1', sc1[0], False)

            # =========== phase 2: relconv (fused QKV) ===========
            sc2 = nc.enter_named_scope('ph2_relconv', False)
            with (
                tc.tile_pool(name='p2g', bufs=3) as gp2,
                tc.tile_pool(name='p2sb', bufs=3) as sb2,
                tc.tile_pool(name='p2S', bufs=1, space='PSUM') as psS,
                tc.tile_pool(name='p2qkv', bufs=1, space='PSUM') as psQ,
                tc.tile_pool(name='p2tr', bufs=1, space='PSUM') as psT,
            ):
                rc_off = 0   # chunk offset
                kv_stores = []
                cc2 = [None, None]
                W_LO = M_LO // WIN
                for w in range(NW):
                    qkv_ps = [psQ.tile([128, 3 * D], f32, tag=f'qkv{i}', name=f'qkv{i}') for i in range(2)]
                    S = psS.tile([128, REL * WIN], f32, tag='S', name='S')
                    # zero the whole S tile: regions are half-bank sized, so
                    # per-region matmul start=True cannot be used safely
                    nc.scalar.activation(out=S[:], in_=zeros2304[:], func=AF.Copy)
                    # (chunk, rel) touch schedule for this window: start/stop
                    # flags per relation region of S
                    touches = []
                    for hh in range(2):
                        nch = rc_chunks[w * 2 + hh]
                        for k in range(nch):
                            for r in rc_sched[rc_off + k]:
                                touches.append((hh, k, r))
                        rc_off += nch
                    rc_off -= rc_chunks[w * 2] + rc_chunks[w * 2 + 1]
                    first_touch = {}
                    last_touch = {}
                    for t in touches:
                        r = t[2]
                        if r not in first_touch:
                            first_touch[r] = t
                        last_touch[r] = t
                    rels_present = sorted(first_touch)
                    for hh in range(2):
                        nch = rc_chunks[w * 2 + hh]
                        co = rc_off
                        gtiles = {}
                        done = 0
                        while done < nch:
                            take = min(nch - done, CALL_MAX)
                            gt = gp2.tile([128, CALL_MAX * D], bf16, tag='g')
                            gi = nc.gpsimd.dma_gather(
                                out_ap=gt[:, :take * D].rearrange('p (c e) -> p c e', e=D),
                                in_ap=hn_tab[hh],
                                idxs_ap=rci[:, (co + done) * 8:(co + done + take) * 8],
                                num_idxs=take * 128, num_idxs_reg=take * 128,
                                elem_size=D)
                            add_dep_helper(gi.ins, cc1[hh].ins, True,
                                           'gather reads allgathered hn')
                            for j in range(take):
                                gtiles[done + j] = (gt, j)
                            done += take
                        for k in range(nch):
                            gt, j = gtiles[k]
                            rels = rc_sched[rc_off + k]
                            if not rels:
                                continue
                            rmin, rmax = rels[0], rels[-1]
                            span = rmax - rmin + 1
                            # one is_eq covers the chunk's whole relation span
                            A = sb2.tile([128, REL * WIN], bf16, tag='A')
                            nc.vector.tensor_tensor(
                                out=A[:, :span * WIN],
                                in0=rck[:, rc_off + k:rc_off + k + 1].to_broadcast([128, span * WIN]),
                                in1=iota2304[:, rmin * WIN:(rmax + 1) * WIN], op=AO.is_equal)
                            for r in rels:
                                nc.tensor.matmul(
                                    out=S[:, r * WIN:(r + 1) * WIN],
                                    lhsT=gt[:, j * D:(j + 1) * D],
                                    rhs=A[:, (r - rmin) * WIN:(r - rmin + 1) * WIN],
                                    start=False,
                                    stop=(last_touch[r] == (hh, k, r)),
                                    skip_group_check=True)
                        rc_off += nch
                    # transforms: batched PSUM->SBUF casts, then per-rel matmuls
                    st_all = sb2.tile([128, REL * WIN], bf16, tag='St')
                    for piece in range(3):
                        nc.vector.tensor_copy(
                            out=st_all[:, piece * 3 * WIN:(piece + 1) * 3 * WIN],
                            in_=S[:, piece * 3 * WIN:(piece + 1) * 3 * WIN])
                    for sub in range(2):
                        for ri, r in enumerate(rels_present):
                            nc.tensor.matmul(out=qkv_ps[sub][:],
                                             lhsT=st_all[:, r * WIN + sub * 128:r * WIN + sub * 128 + 128],
                                             rhs=wrel_sb[:, r * 3 * D:(r + 1) * 3 * D],
                                             start=(ri == 0), stop=False)
                    # self-loop + bias + relu + store
                    for sub in range(2):
                        row0 = w * WIN + sub * 128
                        hnw = sb2.tile([128, D], bf16, tag='hnl')
                        nc.sync.dma_start(out=hnw[:], in_=hn_local[row0:row0 + 128, :])
                        ht_ps = psT.tile([128, 128], bf16, tag='ht')
                        nc.tensor.transpose(out=ht_ps[:], in_=hnw[:], identity=ident[:])
                        ht = sb2.tile([128, 128], bf16, tag='htsb')
                        nc.vector.tensor_copy(out=ht[:], in_=ht_ps[:])
                        nc.tensor.matmul(out=qkv_ps[sub][:], lhsT=ht[:], rhs=wloop_sb[:],
                                         start=(len(rels_present) == 0), stop=True)
                        qkv_sb = sb2.tile([128, 3 * D], bf16, tag='qkvsb')
                        nc.vector.tensor_tensor(out=qkv_sb[:], in0=qkv_ps[sub][:],
                                                in1=bqkv_sb[:], op=AO.add)
                        nc.scalar.activation(out=qkv_sb[:], in_=qkv_sb[:], func=AF.Relu)
                        nc.sync.dma_start(out=q_local[row0:row0 + 128, :], in_=qkv_sb[:, :D])
                        si = nc.sync.dma_start(out=kv_local[row0:row0 + 128, :], in_=qkv_sb[:, D:])
                        kv_stores.append(si)
                    if w == W_LO - 1:
                        cc2[0] = nc.gpsimd.collective_compute(
                            'AllGather', AO.bypass, replica_groups=[list(range(NCORES))],
                            ins=[kv_local[0:M_LO, :]], outs=[kv_lo[:]])
                        for st_i in kv_stores:
                            add_dep_helper(cc2[0].ins, st_i.ins, True, 'cc2a after lo kv stores')
                cc2[1] = nc.gpsimd.collective_compute(
                    'AllGather', AO.bypass, replica_groups=[list(range(NCORES))],
                    ins=[kv_local[M_LO:, :]], outs=[kv_hi[:]])
                for st_i in kv_stores[2 * W_LO:]:
                    add_dep_helper(cc2[1].ins, st_i.ins, True, 'cc2b after hi kv stores')

            nc.leave_named_scope('ph2_relconv', sc2[0], False)

            # =========== phase 3: attention + epilogue ===========
            sc3 = nc.enter_named_scope('ph3_attn', False)
            NCH_MAX = max(at_chunks[2 * s] + at_chunks[2 * s + 1] for s in range(NSW))
            h1_cm = tc.tile_pool(name='h1', bufs=1)
            h1_pool = h1_cm.__enter__()
            h1t = []
            with (
                tc.tile_pool(name='p3g', bufs=3) as gp3,
                tc.tile_pool(name='p3sb', bufs=3) as sb3,
                tc.tile_pool(name='p3at', bufs=2, space='PSUM') as psA,
                tc.tile_pool(name='p3wv', bufs=2, space='PSUM') as psW,
                tc.tile_pool(name='p3ep', bufs=1, space='PSUM') as psE,
            ):
                at_off = 0
                for sw in range(NSW):
                    qwin = sb3.tile([128, D], bf16, tag='qwin')
                    nc.sync.dma_start(out=qwin[:], in_=q_local[sw * 128:(sw + 1) * 128, :])
                    wvz = psW.tile([128, D + HEADS], f32, tag='wvz')
                    nl = at_chunks[sw * 2]
                    nh = at_chunks[sw * 2 + 1]
                    nch = nl + nh
                    # one contiguous gather region per subwindow: lo chunks
                    # [0, nl) then hi chunks [nl, nch)
                    gt = gp3.tile([128, NCH_MAX * 2 * D], bf16, tag='ag')
                    for hh, cnt, coff in ((0, nl, 0), (1, nh, nl)):
                        done = 0
                        while done < cnt:
                            take = min(cnt - done, CALL_MAX)
                            gi = nc.gpsimd.dma_gather(
                                out_ap=gt[:, (coff + done) * 2 * D:(coff + done + take) * 2 * D]
                                    .rearrange('p (c e) -> p c e', e=2 * D),
                                in_ap=kv_tab[hh],
                                idxs_ap=ati[:, (at_off + coff + done) * 8:(at_off + coff + done + take) * 8],
                                num_idxs=take * 128, num_idxs_reg=take * 128,
                                elem_size=2 * D)
                            add_dep_helper(gi.ins, cc2[hh].ins, True,
                                           'gather reads allgathered kv')
                            done += take
                    # process chunks in pairs (last one may be a singleton)
                    p = 0
                    first = True
                    while p < nch:
                        cn = 2 if p + 1 < nch else 1
                        ck = at_off + p
                        A2 = sb3.tile([128, 256], bf16, tag='aA')
                        nc.vector.tensor_tensor(
                            out=A2[:, :cn * 128].rearrange('p (c f) -> p c f', f=128),
                            in0=atk[:, ck:ck + cn].rearrange('p (c o) -> p c o', o=1)
                                .to_broadcast([128, cn, 128]),
                            in1=iota256[:, :cn * 128].rearrange('p (c f) -> p c f', f=128),
                            op=AO.is_equal)
                        at_ps = psA.tile([128, 256], bf16, tag='atp')
                        for j in range(cn):
                            nc.tensor.transpose(out=at_ps[:, j * 128:(j + 1) * 128],
                                                in_=A2[:, j * 128:(j + 1) * 128],
                                                identity=ident[:])
                        att2 = sb3.tile([128, 256], bf16, tag='att')
                        nc.vector.tensor_copy(out=att2[:, :cn * 128], in_=at_ps[:, :cn * 128])
                        qd_ps = psA.tile([128, 256], f32, tag='qd')
                        for j in range(cn):
                            nc.tensor.matmul(out=qd_ps[:, j * D:(j + 1) * D],
                                             lhsT=att2[:, j * 128:(j + 1) * 128],
                                             rhs=qwin[:], start=True, stop=True)
                        kv2 = gt[:, p * 2 * D:(p + cn) * 2 * D].rearrange(
                            'p (c z) -> p c z', z=2 * D)
                        kq2 = sb3.tile([128, 256], f32, tag='kq')
                        nc.vector.tensor_tensor(
                            out=kq2[:, :cn * D].rearrange('p (c d) -> p c d', d=D),
                            in0=kv2[:, :, :D],
                            in1=qd_ps[:, :cn * D].rearrange('p (c d) -> p c d', d=D),
                            op=AO.mult)
                        sc2 = sb3.tile([128, 2 * HEADS], bf16, tag='sc')
                        with nc.allow_low_precision(reason='16-elem head dot, fp16 ok'):
                            nc.vector.reduce_sum(
                                out=sc2[:, :cn * HEADS].rearrange('p (g o) -> p g o', o=1),
                                in_=kq2[:, :cn * D].rearrange('p (g e) -> p g e', e=DH),
                                axis=mybir.AxisListType.X)
                        # scores are >= 0 (dot of relu vectors): only the upper
                        # clip can bind: min(s,40)/4 == clip(s/4, 10). The /4
                        # folds into the exp scale; exp is shifted by -5 so
                        # V*exp fits fp16 (wV and z scale together).
                        nc.vector.tensor_tensor(out=sc2[:, :cn * HEADS],
                                                in0=sc2[:, :cn * HEADS],
                                                in1=c40h[:].to_broadcast([128, cn * HEADS]),
                                                op=AO.min)
                        vse2 = sb3.tile([128, 2 * (D + HEADS)], bf16, tag='vse')
                        vv = vse2[:].rearrange('p (c x) -> p c x', x=D + HEADS)
                        nc.scalar.activation(
                            out=vv[:, :cn, D:], in_=sc2[:, :cn * HEADS]
                                .rearrange('p (c h) -> p c h', h=HEADS),
                            func=AF.Exp, bias=neg5[:], scale=0.25)
                        nc.vector.tensor_tensor(
                            out=vv[:, :cn, :D].rearrange('p c (h e) -> p c h e', e=DH),
                            in0=kv2[:, :, D:].rearrange('p c (h e) -> p c h e', e=DH),
                            in1=vv[:, :cn, D:].rearrange('p c (h o) -> p c h o', o=1)
                                .to_broadcast([128, cn, HEADS, DH]),
                            op=AO.mult)
                        for j in range(cn):
                            last = (p + j == nch - 1)
                            nc.tensor.matmul(out=wvz[:], lhsT=A2[:, j * 128:(j + 1) * 128],
                                             rhs=vse2[:, j * (D + HEADS):(j + 1) * (D + HEADS)],
                                             start=first, stop=last)
                            first = False
                        p += cn
                    at_off += nch
                    # epilogue for this subwindow
                    zr = sb3.tile([128, HEADS], f32, tag='zr')
                    nc.vector.tensor_scalar_add(out=zr[:], in0=wvz[:, D:],
                                                scalar1=6.7379470e-09)  # 1e-6 * exp(-5)
                    zrec = sb3.tile([128, HEADS], f32, tag='zrec')
                    nc.vector.reciprocal(out=zrec[:], in_=zr[:])
                    attn = sb3.tile([128, D], bf16, tag='attn')
                    nc.vector.tensor_tensor(
                        out=attn[:].rearrange('p (h e) -> p h e', e=DH),
                        in0=wvz[:, :D].rearrange('p (h e) -> p h e', e=DH),
                        in1=zrec[:].rearrange('p (h o) -> p h o', o=1).to_broadcast([128, HEADS, DH]),
                        op=AO.mult)
                    atr_ps = psE.tile([128, D], bf16, tag='atr')
                    nc.tensor.transpose(out=atr_ps[:], in_=attn[:], identity=ident[:])
                    atr = sb3.tile([128, D], bf16, tag='atrsb')
                    nc.vector.tensor_copy(out=atr[:], in_=atr_ps[:])
                    ho_ps = psE.tile([128, D], f32, tag='ho')
                    nc.tensor.matmul(out=ho_ps[:], lhsT=atr[:], rhs=ow_sb[:], start=True, stop=True)
                    # LN1 (fused): hob = ho+o_b with row-sum accumulated
                    hob = sb3.tile([128, D], f32, tag='hob')
                    mus = sb3.tile([128, 1], f32, tag='mus')
                    nc.vector.scalar_tensor_tensor(out=hob[:], in0=ho_ps[:], scalar=1.0,
                                                   op0=AO.mult, in1=ob_sb[:], op1=AO.add,
                                                   accum_out=mus[:])
                    mu = sb3.tile([128, 1], f32, tag='mu')
                    nc.vector.tensor_scalar_mul(out=mu[:], in0=mus[:], scalar1=1.0 / D)
                    xc = sb3.tile([128, D], f32, tag='xc')
                    nc.vector.scalar_tensor_tensor(out=xc[:], in0=hob[:], scalar=mu[:],
                                                   op0=AO.subtract, in1=ones_d[:], op1=AO.mult)
                    sq = sb3.tile([128, D], f32, tag='sq')
                    vr = sb3.tile([128, 1], f32, tag='vr')
                    nc.scalar.activation(out=sq[:], in_=xc[:], func=AF.Square,
                                         accum_out=vr[:])
                    nc.vector.tensor_scalar(out=vr[:], in0=vr[:], scalar1=1.0 / D,
                                            scalar2=1e-5, op0=AO.mult, op1=AO.add)
                    sd = sb3.tile([128, 1], f32, tag='sd')
                    nc.scalar.activation(out=sd[:], in_=vr[:], func=AF.Sqrt)
                    rsd = sb3.tile([128, 1], f32, tag='rsd')
                    nc.vector.reciprocal(out=rsd[:], in_=sd[:])
                    h1f = sb3.tile([128, D], f32, tag='h1f')
                    nc.vector.scalar_tensor_tensor(out=h1f[:], in0=xc[:], scalar=rsd[:],
                                                   op0=AO.mult, in1=csb['ln1_g'][:], op1=AO.mult)
                    gi, goff = sw2grp[sw]
                    if goff == 0:
                        h1p = h1_pool.tile([128, groups[gi] * D], bf16, tag=f'h1g_{gi}',
                                           name=f'h1g_{gi}')
                        h1t.append(h1p)
                    nc.vector.tensor_tensor(out=h1t[-1][:, goff * D:(goff + 1) * D],
                                            in0=h1f[:], in1=csb['ln1_b'][:], op=AO.add)

            nc.leave_named_scope('ph3_attn', sc3[0], False)
            # =========== phase 4: graphNorm2 stats + finalize ===========
            sc4 = nc.enter_named_scope('ph45_tail', False)
            with (
                tc.tile_pool(name='p4keep', bufs=1) as keep4,
                tc.tile_pool(name='p4sb', bufs=3) as sb4,
                tc.tile_pool(name='p4ps', bufs=1, space='PSUM') as ps4,
                tc.tile_pool(name='p4ps2', bufs=1, space='PSUM') as ps4b,
            ):
                sum2 = ps4.tile([GMAX, D], f32, tag='sum2')
                sq2 = ps4.tile([GMAX, D], f32, tag='sq2')
                iotaGx = {2: iotaGG, 4: iotaG4}
                s0 = 0
                for gi, gsz in enumerate(groups):
                    h1p = h1t[gi]
                    Bg = sb4.tile([128, 4 * GMAX], bf16, tag='B2')
                    nc.vector.tensor_tensor(
                        out=Bg[:, :gsz * GMAX].rearrange('p (c g) -> p c g', g=GMAX),
                        in0=segs[:, s0:s0 + gsz].rearrange('p (c o) -> p c o', o=1)
                            .to_broadcast([128, gsz, GMAX]),
                        in1=iotaGx[gsz][:, :gsz * GMAX].rearrange('p (c g) -> p c g', g=GMAX),
                        op=AO.is_equal)
                    hsq = sb4.tile([128, 4 * D], bf16, tag='h2sq')
                    nc.vector.tensor_tensor(out=hsq[:, :gsz * D], in0=h1p[:], in1=h1p[:],
                                            op=AO.mult)
                    for j in range(gsz):
                        s = s0 + j
                        nc.tensor.matmul(out=sum2[:], lhsT=Bg[:, j * GMAX:(j + 1) * GMAX],
                                         rhs=h1p[:, j * D:(j + 1) * D],
                                         start=(s == 0), stop=(s == NSW - 1))
                        nc.tensor.matmul(out=sq2[:], lhsT=Bg[:, j * GMAX:(j + 1) * GMAX],
                                         rhs=hsq[:, j * D:(j + 1) * D],
                                         start=(s == 0), stop=(s == NSW - 1))
                    s0 += gsz
                mean2 = keep4.tile([GMAX, D], f32)
                nc.vector.tensor_tensor(out=mean2[:], in0=sum2[:],
                                        in1=invc_sb[:].to_broadcast([GMAX, D]), op=AO.mult)
                ex22 = keep4.tile([GMAX, D], f32)
                nc.vector.tensor_tensor(out=ex22[:], in0=sq2[:],
                                        in1=invc_sb[:].to_broadcast([GMAX, D]), op=AO.mult)
                msq2 = keep4.tile([GMAX, D], f32)
                nc.vector.tensor_tensor(out=msq2[:], in0=mean2[:], in1=mean2[:], op=AO.mult)
                nc.vector.tensor_tensor(out=msq2[:], in0=msq2[:], in1=csb['gn2_msfac16'][:], op=AO.mult)
                var2 = keep4.tile([GMAX, D], f32)
                nc.vector.tensor_tensor(out=var2[:], in0=ex22[:], in1=msq2[:], op=AO.subtract)
                nc.vector.tensor_scalar_add(out=var2[:], in0=var2[:], scalar1=1e-6)
                std2 = keep4.tile([GMAX, D], f32)
                nc.scalar.activation(out=std2[:], in_=var2[:], func=AF.Sqrt)
                rstd2 = keep4.tile([GMAX, D], f32)
                nc.vector.reciprocal(out=rstd2[:], in_=std2[:])
                alpha2 = keep4.tile([GMAX, D], f32)
                nc.vector.tensor_tensor(out=alpha2[:], in0=rstd2[:], in1=csb['gn2_w16'][:], op=AO.mult)
                beta2 = keep4.tile([GMAX, D], f32)
                nc.vector.tensor_tensor(out=beta2[:], in0=mean2[:], in1=csb['gn2_ms16'][:], op=AO.mult)
                nc.vector.tensor_tensor(out=beta2[:], in0=beta2[:], in1=alpha2[:], op=AO.mult)
                nc.vector.tensor_tensor(out=beta2[:], in0=csb['gn2_b16'][:], in1=beta2[:], op=AO.subtract)
                alpha2h = keep4.tile([GMAX, D], bf16)
                nc.vector.tensor_copy(out=alpha2h[:], in_=alpha2[:])
                beta2h = keep4.tile([GMAX, D], bf16)
                nc.vector.tensor_copy(out=beta2h[:], in_=beta2[:])

                # =========== phase 5: gn2 apply + FFN + LN2 (sw groups) ===========
                vr_all = keep4.tile([128, NSW], f32)
                xct = []
                s0 = 0
                for gi, gsz in enumerate(groups):
                    h1p = h1t[gi]
                    Bg = sb4.tile([128, 4 * GMAX], bf16, tag='B3')
                    nc.vector.tensor_tensor(
                        out=Bg[:, :gsz * GMAX].rearrange('p (c g) -> p c g', g=GMAX),
                        in0=segs[:, s0:s0 + gsz].rearrange('p (c o) -> p c o', o=1)
                            .to_broadcast([128, gsz, GMAX]),
                        in1=iotaGx[gsz][:, :gsz * GMAX].rearrange('p (c g) -> p c g', g=GMAX),
                        op=AO.is_equal)
                    bt_ps = ps4b.tile([GMAX, 512], bf16, tag='bt2')
                    for j in range(gsz):
                        nc.tensor.transpose(out=bt_ps[:, j * 128:(j + 1) * 128],
                                            in_=Bg[:, j * GMAX:(j + 1) * GMAX],
                                            identity=ident[:])
                    bt = sb4.tile([GMAX, 512], bf16, tag='bt2sb')
                    nc.scalar.activation(out=bt[:, :gsz * 128], in_=bt_ps[:, :gsz * 128],
                                         func=AF.Copy)
                    ab_ps = ps4b.tile([128, 8 * D], f32, tag='bigps', name='ab_ps')
                    for j in range(gsz):
                        nc.tensor.matmul(out=ab_ps[:, j * 2 * D:j * 2 * D + D],
                                         lhsT=bt[:, j * 128:(j + 1) * 128],
                                         rhs=alpha2h[:], start=True, stop=False)
                        nc.tensor.matmul(out=ab_ps[:, j * 2 * D + D:(j + 1) * 2 * D],
                                         lhsT=bt[:, j * 128:(j + 1) * 128],
                                         rhs=beta2h[:], start=True, stop=True)
                    ab_v = ab_ps[:].rearrange('p (c x d) -> p c x d', x=2, d=D)
                    h2 = sb4.tile([128, 4 * D], bf16, tag='h2')
                    h2v = h2[:, :gsz * D].rearrange('p (c d) -> p c d', d=D)
                    h1v = h1p[:].rearrange('p (c d) -> p c d', d=D)
                    nc.vector.tensor_tensor(out=h2v, in0=h1v, in1=ab_v[:, :gsz, 0, :], op=AO.mult)
                    nc.vector.tensor_tensor(out=h2v, in0=h2v, in1=ab_v[:, :gsz, 1, :], op=AO.add)
                    h2t_ps = ps4b.tile([128, 8 * D], bf16, tag='trps', name='h2t_ps')
                    for j in range(gsz):
                        nc.tensor.transpose(out=h2t_ps[:, j * D:(j + 1) * D],
                                            in_=h2[:, j * D:(j + 1) * D], identity=ident[:])
                    h2tt = sb4.tile([128, 4 * D], bf16, tag='h2tsb')
                    nc.scalar.activation(out=h2tt[:, :gsz * D], in_=h2t_ps[:, :gsz * D],
                                         func=AF.Copy)
                    f1_ps = ps4b.tile([128, 8 * D], f32, tag='bigps', name='f1_ps')
                    for j in range(gsz):
                        nc.tensor.matmul(out=f1_ps[:, j * 2 * D:(j + 1) * 2 * D],
                                         lhsT=h2tt[:, j * D:(j + 1) * D],
                                         rhs=ffn1_sb[:], start=True, stop=True)
                    fr = sb4.tile([128, 8 * D], bf16, tag='fr')
                    nc.vector.tensor_tensor(
                        out=fr[:, :gsz * 2 * D].rearrange('p (c x) -> p c x', x=2 * D),
                        in0=f1_ps[:, :gsz * 2 * D].rearrange('p (c x) -> p c x', x=2 * D),
                        in1=ffn1b_sb[:].rearrange('p (o x) -> p o x', o=1)
                            .to_broadcast([128, gsz, 2 * D]),
                        op=AO.add)
                    nc.vector.tensor_scalar_max(out=fr[:, :gsz * 2 * D],
                                                in0=fr[:, :gsz * 2 * D], scalar1=0.0)
                    frt_ps = ps4b.tile([128, 8 * D], bf16, tag='trps', name='frt_ps')
                    for j in range(2 * gsz):
                        nc.tensor.transpose(out=frt_ps[:, j * D:(j + 1) * D],
                                            in_=fr[:, j * D:(j + 1) * D], identity=ident[:])
                    frt = sb4.tile([128, 8 * D], bf16, tag='frtsb')
                    nc.scalar.activation(out=frt[:, :gsz * 2 * D], in_=frt_ps[:, :gsz * 2 * D],
                                         func=AF.Copy)
                    h3_ps = ps4b.tile([128, 4 * D], f32, tag='h3')
                    for j in range(gsz):
                        nc.tensor.matmul(out=h3_ps[:, j * D:(j + 1) * D],
                                         lhsT=frt[:, j * 2 * D:j * 2 * D + D],
                                         rhs=ffn2_sb[:, :D], start=True, stop=False)
                        nc.tensor.matmul(out=h3_ps[:, j * D:(j + 1) * D],
                                         lhsT=frt[:, j * 2 * D + D:(j + 1) * 2 * D],
                                         rhs=ffn2_sb[:, D:], start=False, stop=True)
                    h3b = sb4.tile([128, 4 * D], f32, tag='h3b')
                    nc.vector.tensor_tensor(
                        out=h3b[:, :gsz * D].rearrange('p (c d) -> p c d', d=D),
                        in0=h3_ps[:, :gsz * D].rearrange('p (c d) -> p c d', d=D),
                        in1=ffn2b_sb[:].rearrange('p (o d) -> p o d', o=1)
                            .to_broadcast([128, gsz, D]),
                        op=AO.add)
                    # LN2 stats (sqrt deferred and batched)
                    mu = sb4.tile([128, 4], f32, tag='mu2')
                    nc.vector.reduce_sum(out=mu[:, :gsz].rearrange('p (c o) -> p c o', o=1),
                                         in_=h3b[:, :gsz * D].rearrange('p (c d) -> p c d', d=D),
                                         axis=mybir.AxisListType.X)
                    nc.vector.tensor_scalar_mul(out=mu[:, :gsz], in0=mu[:, :gsz], scalar1=1.0 / D)
                    xc = keep4.tile([128, 4 * D], f32, tag=f'xc2_{gi}', name=f'xc2_{gi}')
                    nc.vector.tensor_tensor(
                        out=xc[:, :gsz * D].rearrange('p (c d) -> p c d', d=D),
                        in0=h3b[:, :gsz * D].rearrange('p (c d) -> p c d', d=D),
                        in1=mu[:, :gsz].rearrange('p (c o) -> p c o', o=1).to_broadcast([128, gsz, D]),
                        op=AO.subtract)
                    sq = sb4.tile([128, 4 * D], f32, tag='sq2w')
                    nc.vector.tensor_tensor(out=sq[:, :gsz * D], in0=xc[:, :gsz * D],
                                            in1=xc[:, :gsz * D], op=AO.mult)
                    nc.vector.reduce_sum(
                        out=vr_all[:, s0:s0 + gsz].rearrange('p (c o) -> p c o', o=1),
                        in_=sq[:, :gsz * D].rearrange('p (c d) -> p c d', d=D),
                        axis=mybir.AxisListType.X)
                    xct.append(xc)
                    s0 += gsz
                # batched 1/sqrt(var/D + eps) for all subwindows
                nc.vector.tensor_scalar(out=vr_all[:], in0=vr_all[:], scalar1=1.0 / D,
                                        scalar2=1e-5, op0=AO.mult, op1=AO.add)
                sd_all = keep4.tile([128, NSW], f32)
                nc.scalar.activation(out=sd_all[:], in_=vr_all[:], func=AF.Sqrt)
                rsd_all = keep4.tile([128, NSW], f32)
                nc.vector.reciprocal(out=rsd_all[:], in_=sd_all[:])
                s0 = 0
                for gi, gsz in enumerate(groups):
                    ov = sb4.tile([128, 4 * D], f32, tag='ov')
                    ovv = ov[:, :gsz * D].rearrange('p (c d) -> p c d', d=D)
                    nc.vector.tensor_tensor(
                        out=ovv, in0=xct[gi][:, :gsz * D].rearrange('p (c d) -> p c d', d=D),
                        in1=rsd_all[:, s0:s0 + gsz].rearrange('p (c o) -> p c o', o=1)
                            .to_broadcast([128, gsz, D]),
                        op=AO.mult)
                    nc.vector.tensor_tensor(
                        out=ovv, in0=ovv,
                        in1=csb['ln2_g'][:].rearrange('p (o d) -> p o d', o=1)
                            .to_broadcast([128, gsz, D]),
                        op=AO.mult)
                    nc.vector.tensor_tensor(
                        out=ovv, in0=ovv,
                        in1=csb['ln2_b'][:].rearrange('p (o d) -> p o d', o=1)
                            .to_broadcast([128, gsz, D]),
                        op=AO.add)
                    for j in range(gsz):
                        nc.sync.dma_start(
                            out=out_sl[(s0 + j) * 128:(s0 + j + 1) * 128, :],
                            in_=ov[:, j * D:(j + 1) * D])
                    s0 += gsz
            nc.leave_named_scope('ph45_tail', sc4[0], False)
            h1_cm.__exit__(None, None, None)
            if debug:
                nc.sync.dma_start(out=hn_dbg[:], in_=hn_local[:])
                nc.sync.dma_start(out=q_dbg[:], in_=q_local[:])
                nc.sync.dma_start(out=kv_dbg[:], in_=kv_local[:])

    nc.finalize()
    return nc


def kernel(**inputs) -> np.ndarray:
    _ensure_hooks()
    from concourse.bass_utils import run_bass_kernel_spmd

    static, in_maps, meta = preprocess(inputs)
    key = tuple(sorted((k, v) for k, v in static.items()))
    if key not in _PROGRAM_CACHE:
        _PROGRAM_CACHE[key] = build_program(static)
    nc = _PROGRAM_CACHE[key]

    trace = os.environ.get("KERNEL_TRACE") == "1"
    res = run_bass_kernel_spmd(nc, in_maps, list(range(NCORES)), trace=trace)
    if trace and res.exec_time_ns:
        print("HW exec time:", res.exec_time_ns, "ns")
    out = np.zeros((N_NODES, D), np.float32)
    for c in range(NCORES):
        n0, n1 = int(meta['n0'][c]), int(meta['n1'][c])
        out[n0:n1] = res.results[c]['out_slice'][:n1 - n0]
    return out


# revision 51
# speedup vs baseline: 1.1866x; 1.0035x over previous
"""GTLayer (relational graph transformer layer) on 8 Trainium2 NeuronCores.

Strategy
--------
Nodes are partitioned across 8 cores in graph-aligned contiguous slices
(graphNorm stays core-local). Edges live with the core that owns dst.
Per core, dst nodes are processed in 256-node windows.

- graphNorm1: slice-local stats (one-pass sum/sumsq via one-hot matmuls),
  normalize, then AllGather hn -> global gather table (bf16).
- RelConv (Q|K|V fused, 384 cols): edges sorted by (window, src-half,
  relation), each (w,half,r) run padded to 128-slot chunks (>=1 pad slot per
  chunk, index 0, key -1). hn[src] rows fetched with the ext-isa dma_gather
  (int16 indices, signed, two base offsets cover the global table).
  Aggregation is one-hot matmuls in bf16 (PSUM accumulates f32).
- Attention: same chunk machinery per (window, half), all matmuls bf16.
- Epilogue per 128 rows: attn = wV/(z+eps), hO = attn@o_w+o_b, LN1,
  graphNorm2 (stats pass over SBUF-resident h1), FFN, LN2 -> output slice.

All per-core variation is in input data (indices/keys); the SPMD program is
identical across cores (chunk counts are max'ed over cores).
"""
import os
import sys
import types
import numpy as np
BF16 = np.float16

NCORES = 8
N_NODES = 100000
N_EDGES = 600000
D = 128
REL = 9
NG = 64
HEADS = 8
DH = 16
WIN = 256          # dst window (2 x 128 subwindows)
GMAX = 16          # max graphs per core
CALL_MAX = 8       # max chunks (of 128 slots) per dma_gather call


def _ensure_hooks():
    if "antenv.axon_hooks" not in sys.modules:
        hooks = types.ModuleType("antenv.axon_hooks")
        h = [None]
        hooks.set_axon_ntff_profile_hook = lambda v: h.__setitem__(0, v)
        hooks.get_axon_ntff_profile_hook = lambda: h[0]
        sys.modules["antenv.axon_hooks"] = hooks
        try:
            from trn_agent_boot.trn_boot import _ntff_profile_via_ctypes
            hooks.set_axon_ntff_profile_hook(
                _ntff_profile_via_ctypes("/opt/axon/libaxon_pjrt.so"))
        except Exception:
            pass


# ----------------------------------------------------------------------------
# Host preprocessing
# ----------------------------------------------------------------------------

def _pack_idx16(idx):
    """int16 index array -> [128, n/16] wrapped+replicated layout."""
    n = len(idx)
    assert n % 16 == 0
    blk = idx.reshape(n // 16, 16).T
    return np.tile(blk, (8, 1)).astype(np.int16)


def _layout_slots(order_edges, idx_vals, key_vals, n_chunks):
    """Place edges into n_chunks*128 slots, <=127 real per chunk, last slot of
    each chunk is a pad (idx 0, key -1). Returns (idx int32, key int16)."""
    tot = n_chunks * 128
    idx = np.zeros(tot, np.int32)
    key = np.full(tot, -1, np.int16)
    ne = len(order_edges)
    pos = 0
    ei = 0
    for c in range(n_chunks):
        take = min(127, ne - ei)
        if take > 0:
            sl = slice(c * 128, c * 128 + take)
            idx[sl] = idx_vals[order_edges[ei:ei + take]]
            key[sl] = key_vals[order_edges[ei:ei + take]]
            ei += take
    assert ei == ne, (ei, ne, n_chunks)
    return idx, key


def preprocess(inputs):
    h = np.asarray(inputs['h'], np.float32)
    src = np.asarray(inputs['src']).astype(np.int64)
    dst = np.asarray(inputs['dst']).astype(np.int64)
    et = np.asarray(inputs['etypes']).astype(np.int64)
    seg = np.asarray(inputs['seg']).astype(np.int64)

    # --- graph-aligned node partition ---
    gstart = np.searchsorted(seg, np.arange(NG + 1))  # graph g: [gstart[g], gstart[g+1])
    bounds = [0]
    for c in range(1, NCORES):
        target = c * N_NODES / NCORES
        g = int(np.argmin(np.abs(gstart - target)))
        bounds.append(int(gstart[g]))
    bounds.append(N_NODES)
    n0 = np.array(bounds[:-1]); n1 = np.array(bounds[1:])
    sizes = n1 - n0
    P_NODES = int(np.ceil(sizes.max() / WIN) * WIN)
    NW = P_NODES // WIN
    # lo/hi half-tables (each AllGathered separately so the lo collective can
    # overlap with compute of the hi half). int16 gather reach: rows/2 <= 32768.
    M_LO = (NW // 2) * WIN
    M_HI = P_NODES - M_LO
    ROWS_LO = NCORES * M_LO
    ROWS_HI = NCORES * M_HI
    assert ROWS_LO // 2 <= 32768 and ROWS_HI // 2 <= 32768

    owner = np.searchsorted(n1, np.arange(N_NODES), side='right')
    off_all = np.arange(N_NODES) - n0[owner]

    # --- per-core graph info ---
    g0 = np.searchsorted(gstart, n0, side='right') - 1  # first graph on core
    counts_g = np.diff(gstart).astype(np.float32)

    owner_s = owner[src]
    off_s = off_all[src]
    half = (off_s >= M_LO).astype(np.int64)
    ecore = owner[dst]
    dst_off = dst - n0[ecore]
    w_e = dst_off // WIN
    dl_e = (dst_off % WIN).astype(np.float32)
    idx_rel = np.where(half == 0,
                       owner_s * M_LO + off_s - ROWS_LO // 2,
                       owner_s * M_HI + (off_s - M_LO) - ROWS_HI // 2).astype(np.int32)

    # --- relconv structure: runs (w, half); slots sorted by key=et*256+dl so
    # chunks span relation boundaries (one matmul per (chunk, rel-present)) ---
    ckey = (et * 256 + dst_off % WIN).astype(np.int16)
    rkey = w_e * 2 + half
    rc_counts = np.zeros((NCORES, NW * 2), np.int64)
    for c in range(NCORES):
        m = ecore == c
        rc_counts[c] = np.bincount(rkey[m], minlength=NW * 2)
    rc_chunks = np.ceil(rc_counts / 127.0).max(0).astype(np.int64)

    # --- attention structure: segments (sw128, half) ---
    sw_e = dst_off // 128
    dl128 = (dst_off % 128).astype(np.int16)
    NSW = NW * 2
    akey = sw_e * 2 + half
    at_counts = np.zeros((NCORES, NSW * 2), np.int64)
    for c in range(NCORES):
        m = ecore == c
        at_counts[c] = np.bincount(akey[m], minlength=NSW * 2)
    at_chunks = np.maximum(np.ceil(at_counts / 127.0).max(0), 1).astype(np.int64)

    RC_CHUNKS = int(rc_chunks.sum())
    AT_CHUNKS = int(at_chunks.sum())

    # --- per-core data arrays ---
    # rc_sched[chunk] = tuple of relations present in that chunk on ANY core
    rc_rels = [set() for _ in range(RC_CHUNKS)]
    in_maps = []
    for c in range(NCORES):
        m = np.nonzero(ecore == c)[0]
        rk = rkey[m]
        order = np.argsort(rk * 4096 + ckey[m].astype(np.int64), kind='stable')
        edges_sorted = m[order]
        rk_sorted = rk[order]
        run_start = np.searchsorted(rk_sorted, np.arange(NW * 2))
        run_end = np.searchsorted(rk_sorted, np.arange(NW * 2) + 1)

        rc_idx = np.zeros(RC_CHUNKS * 128, np.int32)
        rc_key = np.full(RC_CHUNKS * 128, -1, np.int16)
        coff = 0
        for q in range(NW * 2):
            nch = int(rc_chunks[q])
            eidx = edges_sorted[run_start[q]:run_end[q]]
            ii, kk = _layout_slots(eidx, idx_rel, ckey, nch)
            rc_idx[coff * 128:(coff + nch) * 128] = ii
            rc_key[coff * 128:(coff + nch) * 128] = kk
            for k in range(nch):
                ee = eidx[k * 127:(k + 1) * 127]
                rc_rels[coff + k].update(int(x) for x in np.unique(et[ee]))
            coff += nch
        assert coff == RC_CHUNKS

        ak = akey[m]
        aorder = np.argsort(ak, kind='stable')
        aedges = m[aorder]
        ak_sorted = ak[aorder]
        astart = np.searchsorted(ak_sorted, np.arange(NSW * 2))
        aend = np.searchsorted(ak_sorted, np.arange(NSW * 2) + 1)
        at_idx = np.zeros(AT_CHUNKS * 128, np.int32)
        at_key = np.full(AT_CHUNKS * 128, -1, np.int16)
        coff = 0
        for q in range(NSW * 2):
            nch = int(at_chunks[q])
            eidx = aedges[astart[q]:aend[q]]
            ii, kk = _layout_slots(eidx, idx_rel, dl128, nch)
            at_idx[coff * 128:(coff + nch) * 128] = ii
            at_key[coff * 128:(coff + nch) * 128] = kk
            coff += nch
        assert coff == AT_CHUNKS

        hs = np.zeros((P_NODES, D), np.float32)
        hs[:sizes[c]] = h[n0[c]:n1[c]]
        segl = np.full(P_NODES, -1.0, np.float32)
        segl[:sizes[c]] = (seg[n0[c]:n1[c]] - g0[c]).astype(np.float32)
        ginc = np.zeros((GMAX, 1), np.float32)
        ng_c = int(seg[n1[c] - 1] - g0[c]) + 1
        assert ng_c <= GMAX
        ginc[:ng_c, 0] = 1.0 / counts_g[g0[c]:g0[c] + ng_c]

        im = {
            'h_slice': hs.astype(BF16),
            'seg_col': segl.reshape(NW * 2, 128).T.copy(),   # [128, NW*2]
            'inv_cnt': ginc,
            'rc_idx': _pack_idx16(rc_idx.astype(np.int16)),
            'rc_key': rc_key.reshape(RC_CHUNKS, 128).T.copy(),  # [128, RC_CHUNKS]
            'at_idx': _pack_idx16(at_idx.astype(np.int16)),
            'at_key': at_key.reshape(AT_CHUNKS, 128).T.copy(),
        }
        in_maps.append(im)

    # --- shared weights (same for all cores) ---
    def A(x):
        return np.ascontiguousarray(np.asarray(x, np.float32))
    Wrel = np.concatenate([
        np.einsum('rb,bio->rio', A(inputs[f'{nm}_coeff']), A(inputs[f'{nm}_basis']))
        for nm in ('q', 'k', 'v')], axis=2)            # [9, 128, 384]
    w_shared = {
        'w_rel': A(Wrel.reshape(REL * D, 3 * D)).astype(BF16),
        'w_loop': np.concatenate([A(inputs[f'{nm}_loop']) for nm in ('q', 'k', 'v')], 1).astype(BF16),
        'b_qkv': np.tile(np.concatenate([A(inputs[f'{nm}_bias']) for nm in ('q', 'k', 'v')])[None, :], (128, 1)),
        'o_w': A(inputs['o_w']).astype(BF16), 'o_b': np.tile(A(inputs['o_b'])[None, :], (128, 1)),
        'ffn1': A(inputs['ffn1_w']).astype(BF16), 'ffn1_b': np.tile(A(inputs['ffn1_b'])[None, :], (128, 1)),
        'ffn2': A(inputs['ffn2_w']).astype(BF16), 'ffn2_b': np.tile(A(inputs['ffn2_b'])[None, :], (128, 1)),
        'ln1_g': np.tile(A(inputs['ln1_g'])[None, :], (128, 1)),
        'ln1_b': np.tile(A(inputs['ln1_b'])[None, :], (128, 1)),
        'ln2_g': np.tile(A(inputs['ln2_g'])[None, :], (128, 1)),
        'ln2_b': np.tile(A(inputs['ln2_b'])[None, :], (128, 1)),
    }
    for nm in ('gn1', 'gn2'):
        w = A(inputs[f'{nm}_w']); b = A(inputs[f'{nm}_b']); ms = A(inputs[f'{nm}_ms'])
        w_shared[f'{nm}_w16'] = np.tile(w[None, :], (GMAX, 1))
        w_shared[f'{nm}_b16'] = np.tile(b[None, :], (GMAX, 1))
        w_shared[f'{nm}_ms16'] = np.tile(ms[None, :], (GMAX, 1))
        w_shared[f'{nm}_msfac16'] = np.tile((ms * (2 - ms))[None, :], (GMAX, 1))
    for im in in_maps:
        im.update(w_shared)

    static = dict(P_NODES=P_NODES, NW=NW, M_LO=M_LO, M_HI=M_HI,
                  rc_chunks=tuple(int(x) for x in rc_chunks),
                  rc_sched=tuple(tuple(sorted(s)) for s in rc_rels),
                  at_chunks=tuple(int(x) for x in at_chunks),
                  RC_CHUNKS=RC_CHUNKS, AT_CHUNKS=AT_CHUNKS)
    meta = dict(n0=n0, n1=n1, sizes=sizes)
    return static, in_maps, meta


# ----------------------------------------------------------------------------
# Bass program
# ----------------------------------------------------------------------------

_PROGRAM_CACHE = {}


def build_program(st):
    import concourse.bass as bass
    import concourse.bacc as bacc
    import concourse.mybir as mybir
    import concourse.tile as tile
    from concourse.tile import TileContext
    from concourse.masks import make_identity
    from bass_rust import add_dep_helper

    P_NODES = st['P_NODES']; NW = st['NW']
    M_LO = st['M_LO']; M_HI = st['M_HI']
    ROWS_LO = NCORES * M_LO; ROWS_HI = NCORES * M_HI
    rc_chunks = st['rc_chunks']; at_chunks = st['at_chunks']
    rc_sched = st['rc_sched']
    RC_CHUNKS = st['RC_CHUNKS']; AT_CHUNKS = st['AT_CHUNKS']
    f32 = mybir.dt.float32
    bf16 = mybir.dt.float16
    i16 = mybir.dt.int16
    AO = mybir.AluOpType
    AF = mybir.ActivationFunctionType

    nc = bacc.Bacc()

    # --- I/O ---
    h_slice = nc.declare_dram_parameter('h_slice', [P_NODES, D], bf16, isOutput=False)
    seg_col = nc.declare_dram_parameter('seg_col', [128, NW * 2], f32, isOutput=False)
    inv_cnt = nc.declare_dram_parameter('inv_cnt', [GMAX, 1], f32, isOutput=False)
    rc_idx = nc.declare_dram_parameter('rc_idx', [128, RC_CHUNKS * 8], i16, isOutput=False)
    rc_keyd = nc.declare_dram_parameter('rc_key', [128, RC_CHUNKS], i16, isOutput=False)
    at_idx = nc.declare_dram_parameter('at_idx', [128, AT_CHUNKS * 8], i16, isOutput=False)
    at_keyd = nc.declare_dram_parameter('at_key', [128, AT_CHUNKS], i16, isOutput=False)
    w_rel = nc.declare_dram_parameter('w_rel', [REL * D, 3 * D], bf16, isOutput=False)
    w_loop = nc.declare_dram_parameter('w_loop', [D, 3 * D], bf16, isOutput=False)
    b_qkv = nc.declare_dram_parameter('b_qkv', [128, 3 * D], f32, isOutput=False)
    o_w = nc.declare_dram_parameter('o_w', [D, D], bf16, isOutput=False)
    o_b = nc.declare_dram_parameter('o_b', [128, D], f32, isOutput=False)
    ffn1 = nc.declare_dram_parameter('ffn1', [D, 2 * D], bf16, isOutput=False)
    ffn1_b = nc.declare_dram_parameter('ffn1_b', [128, 2 * D], f32, isOutput=False)
    ffn2 = nc.declare_dram_parameter('ffn2', [2 * D, D], bf16, isOutput=False)
    ffn2_b = nc.declare_dram_parameter('ffn2_b', [128, D], f32, isOutput=False)
    cdecl = {}
    for nm in ('ln1_g', 'ln1_b', 'ln2_g', 'ln2_b'):
        cdecl[nm] = nc.declare_dram_parameter(nm, [128, D], f32, isOutput=False)
    for nm in ('gn1', 'gn2'):
        for sfx in ('w16', 'b16', 'ms16', 'msfac16'):
            cdecl[f'{nm}_{sfx}'] = nc.declare_dram_parameter(
                f'{nm}_{sfx}', [GMAX, D], f32, isOutput=False)
    out_sl = nc.declare_dram_parameter('out_slice', [P_NODES, D], f32, isOutput=True)

    # --- internal DRAM ---
    hn_local = nc.dram_tensor('hn_local', [P_NODES, D], bf16)
    q_local = nc.dram_tensor('q_local', [P_NODES, D], bf16)
    kv_local = nc.dram_tensor('kv_local', [P_NODES, 2 * D], bf16)
    debug = os.environ.get('KERNEL_DEBUG') == '1'
    if debug:
        hn_dbg = nc.declare_dram_parameter('hn_dbg', [P_NODES, D], bf16, isOutput=True)
        q_dbg = nc.declare_dram_parameter('q_dbg', [P_NODES, D], bf16, isOutput=True)
        kv_dbg = nc.declare_dram_parameter('kv_dbg', [P_NODES, 2 * D], bf16, isOutput=True)
    hn_lo = nc.dram_tensor('hn_lo', [NCORES, M_LO, D], bf16, addr_space='Shared')
    hn_hi = nc.dram_tensor('hn_hi', [NCORES, M_HI, D], bf16, addr_space='Shared')
    kv_lo = nc.dram_tensor('kv_lo', [NCORES, M_LO, 2 * D], bf16, addr_space='Shared')
    kv_hi = nc.dram_tensor('kv_hi', [NCORES, M_HI, 2 * D], bf16, addr_space='Shared')
    # gather base views: idx 0 points at the middle row of each half-table
    hn_tab = [hn_lo[:].rearrange('c p d -> (c p) d')[ROWS_LO // 2:, :],
              hn_hi[:].rearrange('c p d -> (c p) d')[ROWS_HI // 2:, :]]
    kv_tab = [kv_lo[:].rearrange('c p d -> (c p) d')[ROWS_LO // 2:, :],
              kv_hi[:].rearrange('c p d -> (c p) d')[ROWS_HI // 2:, :]]

    with TileContext(nc) as tc:
        with tc.tile_pool(name='const', bufs=1) as cpool:
            # constants
            iota2304 = cpool.tile([128, REL * WIN], i16)
            nc.gpsimd.iota(iota2304[:], pattern=[[1, REL * WIN]], base=0,
                           channel_multiplier=0)
            iotaG = cpool.tile([128, GMAX], f32)
            nc.gpsimd.iota(iotaG[:], pattern=[[1, GMAX]], base=0,
                           channel_multiplier=0, allow_small_or_imprecise_dtypes=True)
            iotaGG = cpool.tile([128, 2 * GMAX], f32)
            nc.gpsimd.iota(iotaGG[:].rearrange('p (c f) -> p c f', f=GMAX),
                           pattern=[[0, 2], [1, GMAX]], base=0,
                           channel_multiplier=0, allow_small_or_imprecise_dtypes=True)
            iotaG4 = cpool.tile([128, 4 * GMAX], f32)
            nc.gpsimd.iota(iotaG4[:].rearrange('p (c f) -> p c f', f=GMAX),
                           pattern=[[0, 4], [1, GMAX]], base=0,
                           channel_multiplier=0, allow_small_or_imprecise_dtypes=True)
            ident = cpool.tile([128, 128], bf16)
            make_identity(nc, ident[:])
            ones1 = cpool.tile([1, 128], f32)
            nc.gpsimd.memset(ones1[:], 1.0)
            neg5 = cpool.tile([128, 1], f32)
            nc.gpsimd.memset(neg5[:], -5.0)
            epsz = cpool.tile([128, 1], f32)
            nc.gpsimd.memset(epsz[:], 6.7379470e-09)   # 1e-6 * exp(-5)
            zeros2304 = cpool.tile([128, REL * WIN], f32)
            nc.gpsimd.memset(zeros2304[:], 0.0)
            eps5 = cpool.tile([128, 1], f32)
            nc.gpsimd.memset(eps5[:], 1e-5)
            invd = cpool.tile([128, 1], f32)
            nc.gpsimd.memset(invd[:], 1.0 / D)
            ones_d = cpool.tile([128, D], f32)
            nc.gpsimd.memset(ones_d[:], 1.0)
            iota256 = cpool.tile([128, 256], i16)       # j % 128 pattern
            nc.gpsimd.iota(iota256[:].rearrange('p (c f) -> p c f', f=128),
                           pattern=[[0, 2], [1, 128]], base=0,
                           channel_multiplier=0)
            c40h = cpool.tile([128, 1], bf16)
            nc.gpsimd.memset(c40h[:], 40.0)

            segs = cpool.tile([128, NW * 2], f32)
            nc.sync.dma_start(out=segs[:], in_=seg_col[:])
            rck = cpool.tile([128, RC_CHUNKS], i16)
            nc.sync.dma_start(out=rck[:], in_=rc_keyd[:])
            atk = cpool.tile([128, AT_CHUNKS], i16)
            nc.sync.dma_start(out=atk[:], in_=at_keyd[:])
            rci = cpool.tile([128, RC_CHUNKS * 8], i16)
            nc.sync.dma_start(out=rci[:], in_=rc_idx[:])
            ati = cpool.tile([128, AT_CHUNKS * 8], i16)
            nc.sync.dma_start(out=ati[:], in_=at_idx[:])

            wrel_sb = cpool.tile([128, REL * 3 * D], bf16)  # r-th block at [:, r*384:(r+1)*384]
            for r in range(REL):
                nc.sync.dma_start(out=wrel_sb[:, r * 3 * D:(r + 1) * 3 * D],
                                  in_=w_rel[r * D:(r + 1) * D, :])
            wloop_sb = cpool.tile([128, 3 * D], bf16)
            nc.sync.dma_start(out=wloop_sb[:], in_=w_loop[:])
            bqkv_sb = cpool.tile([128, 3 * D], f32)
            nc.sync.dma_start(out=bqkv_sb[:], in_=b_qkv[:])
            ow_sb = cpool.tile([D, D], bf16)
            nc.sync.dma_start(out=ow_sb[:], in_=o_w[:])
            ob_sb = cpool.tile([128, D], f32)
            nc.sync.dma_start(out=ob_sb[:], in_=o_b[:])
            ffn1_sb = cpool.tile([D, 2 * D], bf16)
            nc.sync.dma_start(out=ffn1_sb[:], in_=ffn1[:])
            ffn1b_sb = cpool.tile([128, 2 * D], f32)
            nc.sync.dma_start(out=ffn1b_sb[:], in_=ffn1_b[:])
            ffn2_sb = cpool.tile([128, 2 * D], bf16)  # two K-chunks side by side
            nc.sync.dma_start(out=ffn2_sb[:, :D], in_=ffn2[:D, :])
            nc.sync.dma_start(out=ffn2_sb[:, D:], in_=ffn2[D:, :])
            ffn2b_sb = cpool.tile([128, D], f32)
            nc.sync.dma_start(out=ffn2b_sb[:], in_=ffn2_b[:])
            csb = {}
            for nm, dd in cdecl.items():
                t = cpool.tile(list(dd.shape), f32, tag=f'c_{nm}')
                nc.sync.dma_start(out=t[:], in_=dd[:])
                csb[nm] = t
            invc_sb = cpool.tile([GMAX, 1], f32)
            nc.sync.dma_start(out=invc_sb[:], in_=inv_cnt[:])

            NSW = NW * 2  # number of 128-row subwindows
            # subwindow groups of 4 (trailing 2 if NSW % 4): batch tail DVE ops
            groups = [4] * (NSW // 4)
            if NSW % 4:
                groups.append(NSW % 4)
            sw2grp = {}
            s0 = 0
            for gi, gsz in enumerate(groups):
                for j in range(gsz):
                    sw2grp[s0 + j] = (gi, j)
                s0 += gsz

            # =========== phase 1: graphNorm1 ===========
            sc1 = nc.enter_named_scope('ph1_gn1', False)
            with (
                tc.tile_pool(name='p1keep', bufs=1) as keep1,
                tc.tile_pool(name='p1sb', bufs=3) as sb1,
                tc.tile_pool(name='p1ps', bufs=1, space='PSUM') as ps1,
                tc.tile_pool(name='p1ps2', bufs=1, space='PSUM') as ps1b,
            ):
                sum_ps = ps1.tile([GMAX, D], f32, tag='sums')
                sq_ps = ps1.tile([GMAX, D], f32, tag='sqs')
                hwins = []
                for sp in range(NSW // 2):
                    hw = keep1.tile([128, 2 * D], bf16, tag=f'h_{sp}')
                    nc.sync.dma_start(out=hw[:, :D],
                                      in_=h_slice[2 * sp * 128:(2 * sp + 1) * 128, :])
                    nc.sync.dma_start(out=hw[:, D:],
                                      in_=h_slice[(2 * sp + 1) * 128:(2 * sp + 2) * 128, :])
                    B2 = sb1.tile([128, 2 * GMAX], bf16, tag='B1')
                    nc.vector.tensor_tensor(
                        out=B2[:].rearrange('p (c g) -> p c g', g=GMAX),
                        in0=segs[:, 2 * sp:2 * sp + 2].rearrange('p (c o) -> p c o', o=1)
                            .to_broadcast([128, 2, GMAX]),
                        in1=iotaGG[:].rearrange('p (c g) -> p c g', g=GMAX),
                        op=AO.is_equal)
                    hsq = sb1.tile([128, 2 * D], bf16, tag='hsq')
                    nc.vector.tensor_tensor(out=hsq[:], in0=hw[:], in1=hw[:], op=AO.mult)
                    for j in range(2):
                        s = 2 * sp + j
                        nc.tensor.matmul(out=sum_ps[:], lhsT=B2[:, j * GMAX:(j + 1) * GMAX],
                                         rhs=hw[:, j * D:(j + 1) * D],
                                         start=(s == 0), stop=(s == NSW - 1))
                        nc.tensor.matmul(out=sq_ps[:], lhsT=B2[:, j * GMAX:(j + 1) * GMAX],
                                         rhs=hsq[:, j * D:(j + 1) * D],
                                         start=(s == 0), stop=(s == NSW - 1))
                    hwins.append(hw)
                # finalize: alpha/beta [GMAX, D]
                mean = keep1.tile([GMAX, D], f32)
                nc.vector.tensor_tensor(out=mean[:], in0=sum_ps[:],
                                        in1=invc_sb[:].to_broadcast([GMAX, D]), op=AO.mult)
                ex2 = keep1.tile([GMAX, D], f32)
                nc.vector.tensor_tensor(out=ex2[:], in0=sq_ps[:],
                                        in1=invc_sb[:].to_broadcast([GMAX, D]), op=AO.mult)
                msq = keep1.tile([GMAX, D], f32)
                nc.vector.tensor_tensor(out=msq[:], in0=mean[:], in1=mean[:], op=AO.mult)
                nc.vector.tensor_tensor(out=msq[:], in0=msq[:], in1=csb['gn1_msfac16'][:], op=AO.mult)
                var = keep1.tile([GMAX, D], f32)
                nc.vector.tensor_tensor(out=var[:], in0=ex2[:], in1=msq[:], op=AO.subtract)
                nc.vector.tensor_scalar_add(out=var[:], in0=var[:], scalar1=1e-6)
                std = keep1.tile([GMAX, D], f32)
                nc.scalar.activation(out=std[:], in_=var[:], func=AF.Sqrt)
                rstd = keep1.tile([GMAX, D], f32)
                nc.vector.reciprocal(out=rstd[:], in_=std[:])
                alpha1 = keep1.tile([GMAX, D], f32)
                nc.vector.tensor_tensor(out=alpha1[:], in0=rstd[:], in1=csb['gn1_w16'][:], op=AO.mult)
                beta1 = keep1.tile([GMAX, D], f32)
                nc.vector.tensor_tensor(out=beta1[:], in0=mean[:], in1=csb['gn1_ms16'][:], op=AO.mult)
                nc.vector.tensor_tensor(out=beta1[:], in0=beta1[:], in1=alpha1[:], op=AO.mult)
                nc.vector.tensor_tensor(out=beta1[:], in0=csb['gn1_b16'][:], in1=beta1[:], op=AO.subtract)
                alpha1h = keep1.tile([GMAX, D], bf16)
                nc.vector.tensor_copy(out=alpha1h[:], in_=alpha1[:])
                beta1h = keep1.tile([GMAX, D], bf16)
                nc.vector.tensor_copy(out=beta1h[:], in_=beta1[:])
                # apply
                hn_stores = []
                SW_LO = M_LO // 128
                cc1 = [None, None]
                for sp in range(NSW // 2):
                    B2 = sb1.tile([128, 2 * GMAX], bf16, tag='B1b')
                    nc.vector.tensor_tensor(
                        out=B2[:].rearrange('p (c g) -> p c g', g=GMAX),
                        in0=segs[:, 2 * sp:2 * sp + 2].rearrange('p (c o) -> p c o', o=1)
                            .to_broadcast([128, 2, GMAX]),
                        in1=iotaGG[:].rearrange('p (c g) -> p c g', g=GMAX),
                        op=AO.is_equal)
                    bt_ps = ps1b.tile([GMAX, 256], bf16, tag='bt')
                    for j in range(2):
                        nc.tensor.transpose(out=bt_ps[:, j * 128:(j + 1) * 128],
                                            in_=B2[:, j * GMAX:(j + 1) * GMAX],
                                            identity=ident[:])
                    bt = sb1.tile([GMAX, 256], bf16, tag='btsb')
                    nc.scalar.activation(out=bt[:], in_=bt_ps[:], func=AF.Copy)
                    ab_ps = ps1b.tile([128, 4 * D], f32, tag='ab')
                    for j in range(2):
                        nc.tensor.matmul(out=ab_ps[:, j * 2 * D:j * 2 * D + D],
                                         lhsT=bt[:, j * 128:(j + 1) * 128],
                                         rhs=alpha1h[:], start=True, stop=False)
                        nc.tensor.matmul(out=ab_ps[:, j * 2 * D + D:(j + 1) * 2 * D],
                                         lhsT=bt[:, j * 128:(j + 1) * 128],
                                         rhs=beta1h[:], start=True, stop=True)
                    ab_v = ab_ps[:].rearrange('p (c x d) -> p c x d', x=2, d=D)
                    hnw = sb1.tile([128, 2 * D], bf16, tag='hnw')
                    hnv = hnw[:].rearrange('p (c d) -> p c d', d=D)
                    nc.vector.tensor_tensor(out=hnv,
                                            in0=hwins[sp][:].rearrange('p (c d) -> p c d', d=D),
                                            in1=ab_v[:, :, 0, :], op=AO.mult)
                    nc.vector.tensor_tensor(out=hnv, in0=hnv, in1=ab_v[:, :, 1, :], op=AO.add)
                    si0 = nc.sync.dma_start(out=hn_local[2 * sp * 128:(2 * sp + 1) * 128, :],
                                            in_=hnw[:, :D])
                    si1 = nc.sync.dma_start(out=hn_local[(2 * sp + 1) * 128:(2 * sp + 2) * 128, :],
                                            in_=hnw[:, D:])
                    hn_stores.extend([si0, si1])
                    if 2 * sp + 2 == SW_LO:
                        cc1[0] = nc.gpsimd.collective_compute(
                            'AllGather', AO.bypass, replica_groups=[list(range(NCORES))],
                            ins=[hn_local[0:M_LO, :]], outs=[hn_lo[:]])
                        for st_i in hn_stores:
                            add_dep_helper(cc1[0].ins, st_i.ins, True, 'cc1a after lo stores')
                cc1[1] = nc.gpsimd.collective_compute(
                    'AllGather', AO.bypass, replica_groups=[list(range(NCORES))],
                    ins=[hn_local[M_LO:, :]], outs=[hn_hi[:]])
                for st_i in hn_stores[SW_LO:]:
                    add_dep_helper(cc1[1].ins, st_i.ins, True, 'cc1b after hi stores')

            nc.leave_named_scope('ph1_gn1', sc1[0], False)

            # =========== phase 2: relconv (fused QKV) ===========
            sc2 = nc.enter_named_scope('ph2_relconv', False)
            with (
                tc.tile_pool(name='p2g', bufs=6) as gp2,
                tc.tile_pool(name='p2sb', bufs=3) as sb2,
                tc.tile_pool(name='p2S', bufs=1, space='PSUM') as psS,
                tc.tile_pool(name='p2qkv', bufs=1, space='PSUM') as psQ,
                tc.tile_pool(name='p2tr', bufs=1, space='PSUM') as psT,
            ):
                rc_off = 0   # chunk offset
                kv_stores = []
                cc2 = [None, None]
                W_LO = M_LO // WIN
                for w in range(NW):
                    qkv_ps = [psQ.tile([128, 3 * D], f32, tag=f'qkv{i}', name=f'qkv{i}') for i in range(2)]
                    S = psS.tile([128, REL * WIN], f32, tag='S', name='S')
                    # zero the whole S tile: regions are half-bank sized, so
                    # per-region matmul start=True cannot be used safely
                    nc.scalar.activation(out=S[:], in_=zeros2304[:], func=AF.Copy)
                    # (chunk, rel) touch schedule for this window: start/stop
                    # flags per relation region of S
                    touches = []
                    for hh in range(2):
                        nch = rc_chunks[w * 2 + hh]
                        for k in range(nch):
                            for r in rc_sched[rc_off + k]:
                                touches.append((hh, k, r))
                        rc_off += nch
                    rc_off -= rc_chunks[w * 2] + rc_chunks[w * 2 + 1]
                    first_touch = {}
                    last_touch = {}
                    for t in touches:
                        r = t[2]
                        if r not in first_touch:
                            first_touch[r] = t
                        last_touch[r] = t
                    rels_present = sorted(first_touch)
                    for hh in range(2):
                        nch = rc_chunks[w * 2 + hh]
                        co = rc_off
                        gtiles = {}
                        done = 0
                        while done < nch:
                            take = min(nch - done, CALL_MAX)
                            gt = gp2.tile([128, CALL_MAX * D], bf16, tag='g')
                            gi = nc.gpsimd.dma_gather(
                                out_ap=gt[:, :take * D].rearrange('p (c e) -> p c e', e=D),
                                in_ap=hn_tab[hh],
                                idxs_ap=rci[:, (co + done) * 8:(co + done + take) * 8],
                                num_idxs=take * 128, num_idxs_reg=take * 128,
                                elem_size=D)
                            add_dep_helper(gi.ins, cc1[hh].ins, True,
                                           'gather reads allgathered hn')
                            for j in range(take):
                                gtiles[done + j] = (gt, j)
                            done += take
                        for k in range(nch):
                            gt, j = gtiles[k]
                            rels = rc_sched[rc_off + k]
                            if not rels:
                                continue
                            rmin, rmax = rels[0], rels[-1]
                            span = rmax - rmin + 1
                            # one is_eq covers the chunk's whole relation span
                            A = sb2.tile([128, REL * WIN], bf16, tag='A')
                            nc.vector.tensor_tensor(
                                out=A[:, :span * WIN],
                                in0=rck[:, rc_off + k:rc_off + k + 1].to_broadcast([128, span * WIN]),
                                in1=iota2304[:, rmin * WIN:(rmax + 1) * WIN], op=AO.is_equal)
                            for r in rels:
                                nc.tensor.matmul(
                                    out=S[:, r * WIN:(r + 1) * WIN],
                                    lhsT=gt[:, j * D:(j + 1) * D],
                                    rhs=A[:, (r - rmin) * WIN:(r - rmin + 1) * WIN],
                                    start=False,
                                    stop=(last_touch[r] == (hh, k, r)),
                                    skip_group_check=True)
                        rc_off += nch
                    # transforms: batched PSUM->SBUF casts, then per-rel matmuls
                    st_all = sb2.tile([128, REL * WIN], bf16, tag='St')
                    for piece in range(3):
                        nc.vector.tensor_copy(
                            out=st_all[:, piece * 3 * WIN:(piece + 1) * 3 * WIN],
                            in_=S[:, piece * 3 * WIN:(piece + 1) * 3 * WIN])
                    for sub in range(2):
                        for ri, r in enumerate(rels_present):
                            nc.tensor.matmul(out=qkv_ps[sub][:],
                                             lhsT=st_all[:, r * WIN + sub * 128:r * WIN + sub * 128 + 128],
                                             rhs=wrel_sb[:, r * 3 * D:(r + 1) * 3 * D],
                                             start=(ri == 0), stop=False)
                    # self-loop + bias + relu + store
                    for sub in range(2):
                        row0 = w * WIN + sub * 128
                        hnw = sb2.tile([128, D], bf16, tag='hnl')
                        nc.sync.dma_start(out=hnw[:], in_=hn_local[row0:row0 + 128, :])
                        ht_ps = psT.tile([128, 128], bf16, tag='ht')
                        nc.tensor.transpose(out=ht_ps[:], in_=hnw[:], identity=ident[:])
                        ht = sb2.tile([128, 128], bf16, tag='htsb')
                        nc.vector.tensor_copy(out=ht[:], in_=ht_ps[:])
                        nc.tensor.matmul(out=qkv_ps[sub][:], lhsT=ht[:], rhs=wloop_sb[:],
                                         start=(len(rels_present) == 0), stop=True)
                        qkv_sb = sb2.tile([128, 3 * D], bf16, tag='qkvsb')
                        nc.vector.tensor_tensor(out=qkv_sb[:], in0=qkv_ps[sub][:],
                                                in1=bqkv_sb[:], op=AO.add)
                        nc.scalar.activation(out=qkv_sb[:], in_=qkv_sb[:], func=AF.Relu)
                        nc.sync.dma_start(out=q_local[row0:row0 + 128, :], in_=qkv_sb[:, :D])
                        si = nc.sync.dma_start(out=kv_local[row0:row0 + 128, :], in_=qkv_sb[:, D:])
                        kv_stores.append(si)
                    if w == W_LO - 1:
                        cc2[0] = nc.gpsimd.collective_compute(
                            'AllGather', AO.bypass, replica_groups=[list(range(NCORES))],
                            ins=[kv_local[0:M_LO, :]], outs=[kv_lo[:]])
                        for st_i in kv_stores:
                            add_dep_helper(cc2[0].ins, st_i.ins, True, 'cc2a after lo kv stores')
                cc2[1] = nc.gpsimd.collective_compute(
                    'AllGather', AO.bypass, replica_groups=[list(range(NCORES))],
                    ins=[kv_local[M_LO:, :]], outs=[kv_hi[:]])
                for st_i in kv_stores[2 * W_LO:]:
                    add_dep_helper(cc2[1].ins, st_i.ins, True, 'cc2b after hi kv stores')

            nc.leave_named_scope('ph2_relconv', sc2[0], False)

            # =========== phase 3: attention + epilogue ===========
            sc3 = nc.enter_named_scope('ph3_attn', False)
            NCH_MAX = max(at_chunks[2 * s] + at_chunks[2 * s + 1] for s in range(NSW))
            h1_cm = tc.tile_pool(name='h1', bufs=1)
            h1_pool = h1_cm.__enter__()
            h1t = []
            with (
                tc.tile_pool(name='p3g', bufs=8) as gp3,
                tc.tile_pool(name='p3sb', bufs=4) as sb3,
                tc.tile_pool(name='p3at', bufs=2, space='PSUM') as psA,
                tc.tile_pool(name='p3wv', bufs=2, space='PSUM') as psW,
                tc.tile_pool(name='p3ep', bufs=1, space='PSUM') as psE,
            ):
                at_off = 0
                for sw in range(NSW):
                    qwin = sb3.tile([128, D], bf16, tag='qwin')
                    nc.sync.dma_start(out=qwin[:], in_=q_local[sw * 128:(sw + 1) * 128, :])
                    wvz = psW.tile([128, D + HEADS], f32, tag='wvz')
                    nl = at_chunks[sw * 2]
                    nh = at_chunks[sw * 2 + 1]
                    nch = nl + nh
                    # one contiguous gather region per subwindow: lo chunks
                    # [0, nl) then hi chunks [nl, nch)
                    gt = gp3.tile([128, NCH_MAX * 2 * D], bf16, tag='ag')
                    for hh, cnt, coff in ((0, nl, 0), (1, nh, nl)):
                        done = 0
                        while done < cnt:
                            take = min(cnt - done, CALL_MAX)
                            gi = nc.gpsimd.dma_gather(
                                out_ap=gt[:, (coff + done) * 2 * D:(coff + done + take) * 2 * D]
                                    .rearrange('p (c e) -> p c e', e=2 * D),
                                in_ap=kv_tab[hh],
                                idxs_ap=ati[:, (at_off + coff + done) * 8:(at_off + coff + done + take) * 8],
                                num_idxs=take * 128, num_idxs_reg=take * 128,
                                elem_size=2 * D)
                            add_dep_helper(gi.ins, cc2[hh].ins, True,
                                           'gather reads allgathered kv')
                            done += take
                    # process chunks in pairs (last one may be a singleton)
                    p = 0
                    first = True
                    while p < nch:
                        cn = 2 if p + 1 < nch else 1
                        ck = at_off + p
                        A2 = sb3.tile([128, 256], bf16, tag='aA')
                        nc.vector.tensor_tensor(
                            out=A2[:, :cn * 128].rearrange('p (c f) -> p c f', f=128),
                            in0=atk[:, ck:ck + cn].rearrange('p (c o) -> p c o', o=1)
                                .to_broadcast([128, cn, 128]),
                            in1=iota256[:, :cn * 128].rearrange('p (c f) -> p c f', f=128),
                            op=AO.is_equal)
                        at_ps = psA.tile([128, 256], bf16, tag='atp')
                        for j in range(cn):
                            nc.tensor.transpose(out=at_ps[:, j * 128:(j + 1) * 128],
                                                in_=A2[:, j * 128:(j + 1) * 128],
                                                identity=ident[:])
                        att2 = sb3.tile([128, 256], bf16, tag='att')
                        nc.vector.tensor_copy(out=att2[:, :cn * 128], in_=at_ps[:, :cn * 128])
                        qd_ps = psA.tile([128, 256], f32, tag='qd')
                        for j in range(cn):
                            nc.tensor.matmul(out=qd_ps[:, j * D:(j + 1) * D],
                                             lhsT=att2[:, j * 128:(j + 1) * 128],
                                             rhs=qwin[:], start=True, stop=True)
                        kv2 = gt[:, p * 2 * D:(p + cn) * 2 * D].rearrange(
                            'p (c z) -> p c z', z=2 * D)
                        kq2 = sb3.tile([128, 256], f32, tag='kq')
                        nc.vector.tensor_tensor(
                            out=kq2[:, :cn * D].rearrange('p (c d) -> p c d', d=D),
                            in0=kv2[:, :, :D],
                            in1=qd_ps[:, :cn * D].rearrange('p (c d) -> p c d', d=D),
                            op=AO.mult)
                        sc2 = sb3.tile([128, 2 * HEADS], bf16, tag='sc')
                        with nc.allow_low_precision(reason='16-elem head dot, fp16 ok'):
                            nc.vector.reduce_sum(
                                out=sc2[:, :cn * HEADS].rearrange('p (g o) -> p g o', o=1),
                                in_=kq2[:, :cn * D].rearrange('p (g e) -> p g e', e=DH),
                                axis=mybir.AxisListType.X)
                        # scores are >= 0 (dot of relu vectors): only the upper
                        # clip can bind: min(s,40)/4 == clip(s/4, 10). The /4
                        # folds into the exp scale; exp is shifted by -5 so
                        # V*exp fits fp16 (wV and z scale together).
                        nc.vector.tensor_tensor(out=sc2[:, :cn * HEADS],
                                                in0=sc2[:, :cn * HEADS],
                                                in1=c40h[:].to_broadcast([128, cn * HEADS]),
                                                op=AO.min)
                        vse2 = sb3.tile([128, 2 * (D + HEADS)], bf16, tag='vse')
                        vv = vse2[:].rearrange('p (c x) -> p c x', x=D + HEADS)
                        nc.scalar.activation(
                            out=vv[:, :cn, D:], in_=sc2[:, :cn * HEADS]
                                .rearrange('p (c h) -> p c h', h=HEADS),
                            func=AF.Exp, bias=neg5[:], scale=0.25)
                        nc.vector.tensor_tensor(
                            out=vv[:, :cn, :D].rearrange('p c (h e) -> p c h e', e=DH),
                            in0=kv2[:, :, D:].rearrange('p c (h e) -> p c h e', e=DH),
                            in1=vv[:, :cn, D:].rearrange('p c (h o) -> p c h o', o=1)
                                .to_broadcast([128, cn, HEADS, DH]),
                            op=AO.mult)
                        for j in range(cn):
                            last = (p + j == nch - 1)
                            nc.tensor.matmul(out=wvz[:], lhsT=A2[:, j * 128:(j + 1) * 128],
                                             rhs=vse2[:, j * (D + HEADS):(j + 1) * (D + HEADS)],
                                             start=first, stop=last)
                            first = False
                        p += cn
                    at_off += nch
                    # epilogue for this subwindow
                    zr = sb3.tile([128, HEADS], f32, tag='zr')
                    nc.vector.tensor_scalar_add(out=zr[:], in0=wvz[:, D:],
                                                scalar1=6.7379470e-09)  # 1e-6 * exp(-5)
                    zrec = sb3.tile([128, HEADS], f32, tag='zrec')
                    nc.vector.reciprocal(out=zrec[:], in_=zr[:])
                    attn = sb3.tile([128, D], bf16, tag='attn')
                    nc.vector.tensor_tensor(
                        out=attn[:].rearrange('p (h e) -> p h e', e=DH),
                        in0=wvz[:, :D].rearrange('p (h e) -> p h e', e=DH),
                        in1=zrec[:].rearrange('p (h o) -> p h o', o=1).to_broadcast([128, HEADS, DH]),
                        op=AO.mult)
                    atr_ps = psE.tile([128, D], bf16, tag='atr')
                    nc.tensor.transpose(out=atr_ps[:], in_=attn[:], identity=ident[:])
                    atr = sb3.tile([128, D], bf16, tag='atrsb')
                    nc.vector.tensor_copy(out=atr[:], in_=atr_ps[:])
                    ho_ps = psE.tile([128, D], f32, tag='ho')
                    nc.tensor.matmul(out=ho_ps[:], lhsT=atr[:], rhs=ow_sb[:], start=True, stop=True)
                    # LN1 (fused): hob = ho+o_b with row-sum accumulated
                    hob = sb3.tile([128, D], f32, tag='hob')
                    mus = sb3.tile([128, 1], f32, tag='mus')
                    nc.vector.scalar_tensor_tensor(out=hob[:], in0=ho_ps[:], scalar=1.0,
                                                   op0=AO.mult, in1=ob_sb[:], op1=AO.add,
                                                   accum_out=mus[:])
                    mu = sb3.tile([128, 1], f32, tag='mu')
                    nc.vector.tensor_scalar_mul(out=mu[:], in0=mus[:], scalar1=1.0 / D)
                    xc = sb3.tile([128, D], f32, tag='xc')
                    nc.vector.scalar_tensor_tensor(out=xc[:], in0=hob[:], scalar=mu[:],
                                                   op0=AO.subtract, in1=ones_d[:], op1=AO.mult)
                    sq = sb3.tile([128, D], f32, tag='sq')
                    vr = sb3.tile([128, 1], f32, tag='vr')
                    nc.scalar.activation(out=sq[:], in_=xc[:], func=AF.Square,
                                         accum_out=vr[:])
                    nc.vector.tensor_scalar(out=vr[:], in0=vr[:], scalar1=1.0 / D,
                                            scalar2=1e-5, op0=AO.mult, op1=AO.add)
                    sd = sb3.tile([128, 1], f32, tag='sd')
                    nc.scalar.activation(out=sd[:], in_=vr[:], func=AF.Sqrt)
                    rsd = sb3.tile([128, 1], f32, tag='rsd')
                    nc.vector.reciprocal(out=rsd[:], in_=sd[:])
                    h1f = sb3.tile([128, D], f32, tag='h1f')
                    nc.vector.scalar_tensor_tensor(out=h1f[:], in0=xc[:], scalar=rsd[:],
                                                   op0=AO.mult, in1=csb['ln1_g'][:], op1=AO.mult)
                    gi, goff = sw2grp[sw]
                    if goff == 0:
                        h1p = h1_pool.tile([128, groups[gi] * D], bf16, tag=f'h1g_{gi}',
                                           name=f'h1g_{gi}')
                        h1t.append(h1p)
                    nc.vector.tensor_tensor(out=h1t[-1][:, goff * D:(goff + 1) * D],
                                            in0=h1f[:], in1=csb['ln1_b'][:], op=AO.add)

            nc.leave_named_scope('ph3_attn', sc3[0], False)
            # =========== phase 4: graphNorm2 stats + finalize ===========
            sc4 = nc.enter_named_scope('ph45_tail', False)
            with (
                tc.tile_pool(name='p4keep', bufs=1) as keep4,
                tc.tile_pool(name='p4sb', bufs=3) as sb4,
                tc.tile_pool(name='p4ps', bufs=1, space='PSUM') as ps4,
                tc.tile_pool(name='p4ps2', bufs=1, space='PSUM') as ps4b,
            ):
                sum2 = ps4.tile([GMAX, D], f32, tag='sum2')
                sq2 = ps4.tile([GMAX, D], f32, tag='sq2')
                iotaGx = {2: iotaGG, 4: iotaG4}
                s0 = 0
                for gi, gsz in enumerate(groups):
                    h1p = h1t[gi]
                    Bg = sb4.tile([128, 4 * GMAX], bf16, tag='B2')
                    nc.vector.tensor_tensor(
                        out=Bg[:, :gsz * GMAX].rearrange('p (c g) -> p c g', g=GMAX),
                        in0=segs[:, s0:s0 + gsz].rearrange('p (c o) -> p c o', o=1)
                            .to_broadcast([128, gsz, GMAX]),
                        in1=iotaGx[gsz][:, :gsz * GMAX].rearrange('p (c g) -> p c g', g=GMAX),
                        op=AO.is_equal)
                    hsq = sb4.tile([128, 4 * D], bf16, tag='h2sq')
                    nc.vector.tensor_tensor(out=hsq[:, :gsz * D], in0=h1p[:], in1=h1p[:],
                                            op=AO.mult)
                    for j in range(gsz):
                        s = s0 + j
                        nc.tensor.matmul(out=sum2[:], lhsT=Bg[:, j * GMAX:(j + 1) * GMAX],
                                         rhs=h1p[:, j * D:(j + 1) * D],
                                         start=(s == 0), stop=(s == NSW - 1))
                        nc.tensor.matmul(out=sq2[:], lhsT=Bg[:, j * GMAX:(j + 1) * GMAX],
                                         rhs=hsq[:, j * D:(j + 1) * D],
                                         start=(s == 0), stop=(s == NSW - 1))
                    s0 += gsz
                mean2 = keep4.tile([GMAX, D], f32)
                nc.vector.tensor_tensor(out=mean2[:], in0=sum2[:],
                                        in1=invc_sb[:].to_broadcast([GMAX, D]), op=AO.mult)
                ex22 = keep4.tile([GMAX, D], f32)
                nc.vector.tensor_tensor(out=ex22[:], in0=sq2[:],
                                        in1=invc_sb[:].to_broadcast([GMAX, D]), op=AO.mult)
                msq2 = keep4.tile([GMAX, D], f32)
                nc.vector.tensor_tensor(out=msq2[:], in0=mean2[:], in1=mean2[:], op=AO.mult)
                nc.vector.tensor_tensor(out=msq2[:], in0=msq2[:], in1=csb['gn2_msfac16'][:], op=AO.mult)
                var2 = keep4.tile([GMAX, D], f32)
                nc.vector.tensor_tensor(out=var2[:], in0=ex22[:], in1=msq2[:], op=AO.subtract)
                nc.vector.tensor_scalar_add(out=var2[:], in0=var2[:], scalar1=1e-6)
                std2 = keep4.tile([GMAX, D], f32)
                nc.scalar.activation(out=std2[:], in_=var2[:], func=AF.Sqrt)
                rstd2 = keep4.tile([GMAX, D], f32)
                nc.vector.reciprocal(out=rstd2[:], in_=std2[:])
                alpha2 = keep4.tile([GMAX, D], f32)
                nc.vector.tensor_tensor(out=alpha2[:], in0=rstd2[:], in1=csb['gn2_w16'][:], op=AO.mult)
                beta2 = keep4.tile([GMAX, D], f32)
                nc.vector.tensor_tensor(out=beta2[:], in0=mean2[:], in1=csb['gn2_ms16'][:], op=AO.mult)
                nc.vector.tensor_tensor(out=beta2[:], in0=beta2[:], in1=alpha2[:], op=AO.mult)
                nc.vector.tensor_tensor(out=beta2[:], in0=csb['gn2_b16'][:], in1=beta2[:], op=AO.subtract)
                alpha2h = keep4.tile([GMAX, D], bf16)
                nc.vector.tensor_copy(out=alpha2h[:], in_=alpha2[:])
                beta2h = keep4.tile([GMAX, D], bf16)
                nc.vector.tensor_copy(out=beta2h[:], in_=beta2[:])

                # =========== phase 5: gn2 apply + FFN + LN2 (sw groups) ===========
                vr_all = keep4.tile([128, NSW], f32)
                xct = []
                s0 = 0
                for gi, gsz in enumerate(groups):
                    h1p = h1t[gi]
                    Bg = sb4.tile([128, 4 * GMAX], bf16, tag='B3')
                    nc.vector.tensor_tensor(
                        out=Bg[:, :gsz * GMAX].rearrange('p (c g) -> p c g', g=GMAX),
                        in0=segs[:, s0:s0 + gsz].rearrange('p (c o) -> p c o', o=1)
                            .to_broadcast([128, gsz, GMAX]),
                        in1=iotaGx[gsz][:, :gsz * GMAX].rearrange('p (c g) -> p c g', g=GMAX),
                        op=AO.is_equal)
                    bt_ps = ps4b.tile([GMAX, 512], bf16, tag='bt2')
                    for j in range(gsz):
                        nc.tensor.transpose(out=bt_ps[:, j * 128:(j + 1) * 128],
                                            in_=Bg[:, j * GMAX:(j + 1) * GMAX],
                                            identity=ident[:])
                    bt = sb4.tile([GMAX, 512], bf16, tag='bt2sb')
                    nc.scalar.activation(out=bt[:, :gsz * 128], in_=bt_ps[:, :gsz * 128],
                                         func=AF.Copy)
                    ab_ps = ps4b.tile([128, 8 * D], f32, tag='bigps', name='ab_ps')
                    for j in range(gsz):
                        nc.tensor.matmul(out=ab_ps[:, j * 2 * D:j * 2 * D + D],
                                         lhsT=bt[:, j * 128:(j + 1) * 128],
                                         rhs=alpha2h[:], start=True, stop=False)
                        nc.tensor.matmul(out=ab_ps[:, j * 2 * D + D:(j + 1) * 2 * D],
                                         lhsT=bt[:, j * 128:(j + 1) * 128],
                                         rhs=beta2h[:], start=True, stop=True)
                    ab_v = ab_ps[:].rearrange('p (c x d) -> p c x d', x=2, d=D)
                    h2 = sb4.tile([128, 4 * D], bf16, tag='h2')
                    h2v = h2[:, :gsz * D].rearrange('p (c d) -> p c d', d=D)
                    h1v = h1p[:].rearrange('p (c d) -> p c d', d=D)
                    nc.vector.tensor_tensor(out=h2v, in0=h1v, in1=ab_v[:, :gsz, 0, :], op=AO.mult)
                    nc.vector.tensor_tensor(out=h2v, in0=h2v, in1=ab_v[:, :gsz, 1, :], op=AO.add)
                    h2t_ps = ps4b.tile([128, 8 * D], bf16, tag='trps', name='h2t_ps')
                    for j in range(gsz):
                        nc.tensor.transpose(out=h2t_ps[:, j * D:(j + 1) * D],
                                            in_=h2[:, j * D:(j + 1) * D], identity=ident[:])
                    h2tt = sb4.tile([128, 4 * D], bf16, tag='h2tsb')
                    nc.scalar.activation(out=h2tt[:, :gsz * D], in_=h2t_ps[:, :gsz * D],
                                         func=AF.Copy)
                    f1_ps = ps4b.tile([128, 8 * D], f32, tag='bigps', name='f1_ps')
                    for j in range(gsz):
                        nc.tensor.matmul(out=f1_ps[:, j * 2 * D:(j + 1) * 2 * D],
                                         lhsT=h2tt[:, j * D:(j + 1) * D],
                                         rhs=ffn1_sb[:], start=True, stop=True)
                    fr = sb4.tile([128, 8 * D], bf16, tag='fr')
                    nc.vector.tensor_tensor(
                        out=fr[:, :gsz * 2 * D].rearrange('p (c x) -> p c x', x=2 * D),
                        in0=f1_ps[:, :gsz * 2 * D].rearrange('p (c x) -> p c x', x=2 * D),
                        in1=ffn1b_sb[:].rearrange('p (o x) -> p o x', o=1)
                            .to_broadcast([128, gsz, 2 * D]),
                        op=AO.add)
                    nc.vector.tensor_scalar_max(out=fr[:, :gsz * 2 * D],
                                                in0=fr[:, :gsz * 2 * D], scalar1=0.0)
                    frt_ps = ps4b.tile([128, 8 * D], bf16, tag='trps', name='frt_ps')
                    for j in range(2 * gsz):
                        nc.tensor.transpose(out=frt_ps[:, j * D:(j + 1) * D],
                                            in_=fr[:, j * D:(j + 1) * D], identity=ident[:])
                    frt = sb4.tile([128, 8 * D], bf16, tag='frtsb')
                    nc.scalar.activation(out=frt[:, :gsz * 2 * D], in_=frt_ps[:, :gsz * 2 * D],
                                         func=AF.Copy)
                    h3_ps = ps4b.tile([128, 4 * D], f32, tag='h3')
                    for j in range(gsz):
                        nc.tensor.matmul(out=h3_ps[:, j * D:(j + 1) * D],
                                         lhsT=frt[:, j * 2 * D:j * 2 * D + D],
                                         rhs=ffn2_sb[:, :D], start=True, stop=False)
                        nc.tensor.matmul(out=h3_ps[:, j * D:(j + 1) * D],
                                         lhsT=frt[:, j * 2 * D + D:(j + 1) * 2 * D],
                                         rhs=ffn2_sb[:, D:], start=False, stop=True)
                    h3b = sb4.tile([128, 4 * D], f32, tag='h3b')
                    nc.vector.tensor_tensor(
                        out=h3b[:, :gsz * D].rearrange('p (c d) -> p c d', d=D),
                        in0=h3_ps[:, :gsz * D].rearrange('p (c d) -> p c d', d=D),
                        in1=ffn2b_sb[:].rearrange('p (o d) -> p o d', o=1)
                            .to_broadcast([128, gsz, D]),
                        op=AO.add)
                    # LN2 stats (sqrt deferred and batched)
                    mu = sb4.tile([128, 4], f32, tag='mu2')
                    nc.vector.reduce_sum(out=mu[:, :gsz].rearrange('p (c o) -> p c o', o=1),
                                         in_=h3b[:, :gsz * D].rearrange('p (c d) -> p c d', d=D),
                                         axis=mybir.AxisListType.X)
                    nc.vector.tensor_scalar_mul(out=mu[:, :gsz], in0=mu[:, :gsz], scalar1=1.0 / D)
                    xc = keep4.tile([128, 4 * D], f32, tag=f'xc2_{gi}', name=f'xc2_{gi}')
                    nc.vector.tensor_tensor(
                        out=xc[:, :gsz * D].rearrange('p (c d) -> p c d', d=D),
                        in0=h3b[:, :gsz * D].rearrange('p (c d) -> p c d', d=D),
                        in1=mu[:, :gsz].rearrange('p (c o) -> p c o', o=1).to_broadcast([128, gsz, D]),
                        op=AO.subtract)
                    sq = sb4.tile([128, 4 * D], f32, tag='sq2w')
                    nc.vector.tensor_tensor(out=sq[:, :gsz * D], in0=xc[:, :gsz * D],
                                            in1=xc[:, :gsz * D], op=AO.mult)
                    nc.vector.reduce_sum(
                        out=vr_all[:, s0:s0 + gsz].rearrange('p (c o) -> p c o', o=1),
                        in_=sq[:, :gsz * D].rearrange('p (c d) -> p c d', d=D),
                        axis=mybir.AxisListType.X)
                    xct.append(xc)
                    s0 += gsz
                # batched 1/sqrt(var/D + eps) for all subwindows
                nc.vector.tensor_scalar(out=vr_all[:], in0=vr_all[:], scalar1=1.0 / D,
                                        scalar2=1e-5, op0=AO.mult, op1=AO.add)
                sd_all = keep4.tile([128, NSW], f32)
                nc.scalar.activation(out=sd_all[:], in_=vr_all[:], func=AF.Sqrt)
                rsd_all = keep4.tile([128, NSW], f32)
                nc.vector.reciprocal(out=rsd_all[:], in_=sd_all[:])
                s0 = 0
                for gi, gsz in enumerate(groups):
                    ov = sb4.tile([128, 4 * D], f32, tag='ov')
                    ovv = ov[:, :gsz * D].rearrange('p (c d) -> p c d', d=D)
                    nc.vector.tensor_tensor(
                        out=ovv, in0=xct[gi][:, :gsz * D].rearrange('p (c d) -> p c d', d=D),
                        in1=rsd_all[:, s0:s0 + gsz].rearrange('p (c o) -> p c o', o=1)
                            .to_broadcast([128, gsz, D]),
                        op=AO.mult)
                    nc.vector.tensor_tensor(
                        out=ovv, in0=ovv,
                        in1=csb['ln2_g'][:].rearrange('p (o d) -> p o d', o=1)
                            .to_broadcast([128, gsz, D]),
                        op=AO.mult)
                    nc.vector.tensor_tensor(
                        out=ovv, in0=ovv,
                        in1=csb['ln2_b'][:].rearrange('p (o d) -> p o d', o=1)
                            .to_broadcast([128, gsz, D]),
                        op=AO.add)
                    for j in range(gsz):
                        nc.sync.dma_start(
                            out=out_sl[(s0 + j) * 128:(s0 + j + 1) * 128, :],
                            in_=ov[:, j * D:(j + 1) * D])
                    s0 += gsz
            nc.leave_named_scope('ph45_tail', sc4[0], False)
            h1_cm.__exit__(None, None, None)
            if debug:
                nc.sync.dma_start(out=hn_dbg[:], in_=hn_local[:])
                nc.sync.dma_start(out=q_dbg[:], in_=q_local[:])
                nc.sync.dma_start(out=kv_dbg[:], in_=kv_local[:])

    nc.finalize()
    return nc


def kernel(**inputs) -> np.ndarray:
    _ensure_hooks()
    from concourse.bass_utils import run_bass_kernel_spmd

    static, in_maps, meta = preprocess(inputs)
    key = tuple(sorted((k, v) for k, v in static.items()))
    if key not in _PROGRAM_CACHE:
        _PROGRAM_CACHE[key] = build_program(static)
    nc = _PROGRAM_CACHE[key]

    trace = os.environ.get("KERNEL_TRACE") == "1"
    res = run_bass_kernel_spmd(nc, in_maps, list(range(NCORES)), trace=trace)
    if trace and res.exec_time_ns:
        print("HW exec time:", res.exec_time_ns, "ns")
    out = np.zeros((N_NODES, D), np.float32)
    for c in range(NCORES):
        n0, n1 = int(meta['n0'][c]), int(meta['n1'][c])
        out[n0:n1] = res.results[c]['out_slice'][:n1 - n0]
    return out
